# revision 1
# baseline (speedup 1.0000x reference)
"""Differential-attention + GroupNorm Trainium2 kernel, 8-core head-parallel.

Problem (hardcoded):
  q, k: [1, 32, 2048, 64] f32 ; v: [1, 16, 2048, 128] f32
  lambda_q1/k1/q2/k2: [64] f32 ; gn_weight/gn_bias: [2048] f32
  out:  [1, 2048, 2048] f32

Sharding: 2 v-heads (= 4 q/k heads) per core across 8 cores. Each core
computes, for each of its v-heads: ghostmax attention w0 - lambda*w1, the
AV product, and the per-head GroupNorm. Scores and AV run in a transposed
layout (keys on partitions, queries free); the small O^T result is
transposed back on the PE so softmax denominators and GroupNorm apply as
cheap per-partition scalars. Host only reshapes/casts (sharding).

Device inputs per core:
  qt   [2, 64, 4096]  bf16 : per v-head, q0^T || q1^T along free dim
  kt   [2, 64, 4096]  bf16 : k0^T || k1^T
  v    [2, 2048, 128] bf16
  lam  [1, 256]       f32  : lambda_q1 | lambda_k1 | lambda_q2 | lambda_k2
  wq   [2, 128, 16]   f32  : gn_weight per (head, q-tile, q%128)
  bq   [2, 128, 16]   f32  : gn_bias * (1-LAMBDA_INIT), same layout
Output:
  out  [2, 128, 2048] f32  : per head, 16 q-tiles of [128 q, 128 d]
                             at columns [128*tt : 128*(tt+1)]
"""
import math
import os
import numpy as np
import ml_dtypes

import concourse.bass as bass
import concourse.mybir as mybir
import concourse.tile as tile
from concourse import bacc
from concourse.bass_utils import run_bass_kernel_spmd
from concourse.masks import make_identity

F32 = mybir.dt.float32
FP16 = mybir.dt.float16
BF16 = mybir.dt.bfloat16
AF = mybir.ActivationFunctionType
ALU = mybir.AluOpType

S = 2048          # sequence length (keys and queries)
D = 64            # head dim of q/k
DV = 128          # head dim of v
HQ = 16           # number of v-heads
NCORE = 8
VH = HQ // NCORE  # v-heads per core = 2
QP = 512          # queries per pass
NPASS = S // QP   # 2
NCH = S // 128    # 16 key chunks
NQT = QP // 128   # 8 q-tiles per pass
LAMBDA_INIT = 0.8
EPS = 1e-5
SCALE = 1.0 / math.sqrt(D)

_PROGRAM = None


def _build_program():
    nc = bacc.Bacc("TRN2", target_bir_lowering=False, debug=False,
                   num_devices=NCORE)
    qt_d = nc.dram_tensor("qt", [VH, D, 2 * S], BF16, kind="ExternalInput").ap()
    kt_d = nc.dram_tensor("kt", [VH, D, 2 * S], BF16, kind="ExternalInput").ap()
    v_d = nc.dram_tensor("v", [VH, S, DV], BF16, kind="ExternalInput").ap()
    lam_d = nc.dram_tensor("lam", [1, 4 * D], F32, kind="ExternalInput").ap()
    wq_d = nc.dram_tensor("wq", [VH, 128, NCH], F32, kind="ExternalInput").ap()
    bq_d = nc.dram_tensor("bq", [VH, 128, NCH], F32, kind="ExternalInput").ap()
    out_d = nc.dram_tensor("out", [VH, 128, S], F32, kind="ExternalOutput").ap()

    def mm(out, lhsT, rhs, start, stop, n_split=512):
        n = rhs.shape[-1]
        for j in range(0, n, n_split):
            e = min(j + n_split, n)
            nc.tensor.matmul(out[:, j:e], lhsT, rhs[:, j:e],
                             start=start, stop=stop)

    with tile.TileContext(nc) as tc:
        with tc.tile_pool(name="const", bufs=1) as const, \
             tc.tile_pool(name="inp", bufs=1) as inp, \
             tc.tile_pool(name="acc", bufs=2) as accp, \
             tc.tile_pool(name="ework", bufs=8) as ework, \
             tc.tile_pool(name="work", bufs=1) as work, \
             tc.tile_pool(name="oct", bufs=2) as octp, \
             tc.tile_pool(name="ps", bufs=2, space="PSUM") as ps:

            ones = const.tile([128, 128], BF16)
            nc.gpsimd.memset(ones[:], 1.0)
            ident = const.tile([128, 128], F32, tag="ident")
            make_identity(nc, ident)

            # ---- inputs ----
            qts, kts, vts, wqs, bqs = [], [], [], [], []
            for h in range(VH):
                qt = inp.tile([D, 2 * S], BF16, tag=f"qt{h}")
                kt = inp.tile([D, 2 * S], BF16, tag=f"kt{h}")
                nc.sync.dma_start(qt[:], qt_d[h])
                nc.sync.dma_start(kt[:], kt_d[h])
                qts.append(qt)
                kts.append(kt)
                vrow = []
                for c in range(NCH):
                    vc = inp.tile([128, DV], BF16, tag=f"v{h}_{c}")
                    nc.sync.dma_start(vc[:], v_d[h, c * 128:(c + 1) * 128, :])
                    vrow.append(vc)
                vts.append(vrow)
                wqt = inp.tile([128, NCH], F32, tag=f"wq{h}")
                bqt = inp.tile([128, NCH], F32, tag=f"bq{h}")
                nc.sync.dma_start(wqt[:], wq_d[h])
                nc.sync.dma_start(bqt[:], bq_d[h])
                wqs.append(wqt)
                bqs.append(bqt)

            lam = inp.tile([1, 4 * D], F32, tag="lam")
            nc.sync.dma_start(lam[:], lam_d[:])

            # ---- lambda_full = exp(lq1.lk1) - exp(lq2.lk2) + 0.8 -> [128,1]
            scr = work.tile([1, D], F32, tag="lscr")
            s12 = work.tile([1, 2], F32, tag="ls12")
            nc.vector.tensor_tensor(scr[:], lam[:, 0:D], lam[:, D:2 * D],
                                    ALU.mult)
            nc.vector.tensor_reduce(s12[:, 0:1], scr[:],
                                    mybir.AxisListType.X, ALU.add)
            nc.vector.tensor_tensor(scr[:], lam[:, 2 * D:3 * D],
                                    lam[:, 3 * D:4 * D], ALU.mult)
            nc.vector.tensor_reduce(s12[:, 1:2], scr[:],
                                    mybir.AxisListType.X, ALU.add)
            e12 = work.tile([1, 2], F32, tag="le12")
            nc.scalar.activation(e12[:], s12[:], AF.Exp)
            lamf = work.tile([1, 1], F32, tag="lamf")
            nc.vector.tensor_tensor(lamf[:], e12[:, 0:1], e12[:, 1:2],
                                    ALU.subtract)
            nc.vector.tensor_scalar(lamf[:], lamf[:], LAMBDA_INIT, None, ALU.add)
            # hi/lo bf16 split for an exact fp32 broadcast through the PE
            lhi = work.tile([1, 1], BF16, tag="lhi")
            nc.vector.tensor_copy(lhi[:], lamf[:])
            llo = work.tile([1, 1], F32, tag="llo")
            nc.vector.tensor_tensor(llo[:], lamf[:], lhi[:], ALU.subtract)
            llob = work.tile([1, 1], BF16, tag="llob")
            nc.vector.tensor_copy(llob[:], llo[:])
            lam_ps = ps.tile([128, QP], F32, tag="pab")  # borrow pab banks
            # warm-up matmuls: keep PE busy early so HAM reaches full clock
            wsc = const.tile([128, 512], BF16, tag="wsc")
            nc.gpsimd.memset(wsc[:], 0.5)
            for _w in range(6):
                nc.tensor.matmul(lam_ps[:, 0:512], ones[:], wsc[:],
                                 start=True, stop=True)
            nc.tensor.matmul(lam_ps[:, 0:1], ones[0:1, :], lhi[:],
                             start=True, stop=False)
            nc.tensor.matmul(lam_ps[:, 0:1], ones[0:1, :], llob[:],
                             start=False, stop=True)
            neglamv = const.tile([128, 1], F32, tag="neglamv")
            nc.vector.tensor_scalar(neglamv[:], lam_ps[:, 0:1], -1.0, None,
                                    ALU.mult)

            inv_n = 1.0 / float(S * DV)
            means2 = work.tile([1, VH], F32, tag="means2")
            vars2 = work.tile([1, VH], F32, tag="vars2")
            octs = []

            # ---- main per-head pipeline (epilogues deferred one pass) ----
            def make_epilogue(h, qp, qsl, acc, o0, o1, oct_t, stats):
                def epi():
                    # per-q-tile denominators: 16 single-column ones-matmuls
                    accb = work.tile([128, 2 * QP], BF16, tag="accb")
                    nc.vector.tensor_copy(accb[:], acc[:])
                    dt = ps.tile([128, 2 * NQT], F32, tag="pab")
                    for t in range(2 * NQT):
                        nc.tensor.matmul(dt[:, t:t + 1],
                                         accb[:, t * 128:(t + 1) * 128],
                                         ones[:, 0:1], start=True, stop=True)
                    rt = work.tile([128, 2 * NQT], F32, tag="rt")
                    nc.vector.reciprocal(rt[:], dt[:])
                    r1l = work.tile([128, NQT], F32, tag="r1l")
                    nc.vector.tensor_scalar(r1l[:], rt[:, NQT:2 * NQT],
                                            neglamv[:], None, ALU.mult)
                    r0q = work.tile([128, QP], F32, tag="r0q")
                    r1q = work.tile([128, QP], F32, tag="r1q")
                    nc.vector.tensor_copy(
                        r0q[:].rearrange("p (t d) -> p t d", t=NQT),
                        rt[:, 0:NQT].broadcast_to([128, NQT, 128]))
                    nc.vector.tensor_copy(
                        r1q[:].rearrange("p (t d) -> p t d", t=NQT),
                        r1l[:].broadcast_to([128, NQT, 128]))
                    o0s = work.tile([128, QP], F32, tag="o0s")
                    o1s = work.tile([128, QP], F32, tag="o1s")
                    nc.vector.tensor_copy(o0s[:], o0[:])
                    nc.vector.tensor_copy(o1s[:], o1[:])
                    ot0r = ps.tile([128, QP], F32, tag="o0")
                    ot1r = ps.tile([128, QP], F32, tag="o1")
                    for t in range(NQT):
                        tsl = slice(t * 128, (t + 1) * 128)
                        nc.tensor.transpose(ot0r[:, tsl], o0s[:, tsl], ident[:])
                        nc.tensor.transpose(ot1r[:, tsl], o1s[:, tsl], ident[:])
                    t0q = work.tile([128, QP], F32, tag="t0q")
                    t1q = work.tile([128, QP], F32, tag="t1q")
                    nc.vector.tensor_tensor(t0q[:], ot0r[:], r0q[:], ALU.mult)
                    nc.vector.tensor_tensor(t1q[:], ot1r[:], r1q[:], ALU.mult)
                    nc.vector.tensor_tensor(oct_t[:, qsl], t0q[:], t1q[:],
                                            ALU.add)
                    nc.vector.tensor_reduce(stats[:, qp:qp + 1],
                                            oct_t[:, qsl],
                                            mybir.AxisListType.X, ALU.add)
                    scr2 = work.tile([128, QP], F32, tag="scr2")
                    nc.vector.tensor_tensor(scr2[:], oct_t[:, qsl],
                                            oct_t[:, qsl], ALU.mult)
                    nc.vector.tensor_reduce(
                        stats[:, NPASS + qp:NPASS + qp + 1], scr2[:],
                        mybir.AxisListType.X, ALU.add)
                return epi

            def finish_head(h, oct_t, stats):
                def fin():
                    octs.append(oct_t)
                    s_all = work.tile([128, 2], F32, tag="s_all")
                    nc.vector.tensor_reduce(s_all[:, 0:1], stats[:, 0:NPASS],
                                            mybir.AxisListType.X, ALU.add)
                    nc.vector.tensor_reduce(s_all[:, 1:2],
                                            stats[:, NPASS:2 * NPASS],
                                            mybir.AxisListType.X, ALU.add)
                    tot = work.tile([1, 2], F32, tag="tot")
                    nc.gpsimd.tensor_reduce(tot[:], s_all[:],
                                            mybir.AxisListType.C, ALU.add)
                    mss = work.tile([1, 2], F32, tag="mss")
                    nc.vector.tensor_scalar(mss[:], tot[:], inv_n, None,
                                            ALU.mult)
                    nc.vector.tensor_copy(means2[:, h:h + 1], mss[:, 0:1])
                    var = work.tile([1, 1], F32, tag="var")
                    nc.vector.tensor_tensor(var[:], mss[:, 0:1], mss[:, 0:1],
                                            ALU.mult)
                    nc.vector.tensor_tensor(var[:], mss[:, 1:2], var[:],
                                            ALU.subtract)
                    nc.vector.tensor_scalar(vars2[:, h:h + 1], var[:], EPS,
                                            None, ALU.add)
                return fin

            pending = []
            head_oct = {}
            for h in range(VH):
                oct_t = octp.tile([128, S], F32, tag="oct")
                stats = work.tile([128, 2 * NPASS], F32, tag="stats")
                head_oct[h] = (oct_t, stats)
                for qp in range(NPASS):
                    qsl = slice(qp * QP, (qp + 1) * QP)
                    q1sl = slice(2048 + qp * QP, 2048 + (qp + 1) * QP)
                    acc = accp.tile([128, 2 * QP], FP16, tag="acc")
                    o0 = ps.tile([128, QP], F32, tag="o0")
                    o1 = ps.tile([128, QP], F32, tag="o1")
                    for c in range(NCH):
                        csl = slice(c * 128, (c + 1) * 128)
                        c1sl = slice(2048 + c * 128, 2048 + (c + 1) * 128)
                        pab = ps.tile([128, 2 * QP], F32, tag="pab")
                        mm(pab[:, 0:QP], kts[h][:, csl], qts[h][:, qsl],
                           True, True)
                        mm(pab[:, QP:2 * QP], kts[h][:, c1sl],
                           qts[h][:, q1sl], True, True)
                        eab = ework.tile([128, 2 * QP], BF16, tag="eab")
                        nc.scalar.activation(eab[:], pab[:], AF.Exp,
                                             scale=SCALE)
                        if c == 0:
                            # seed 1/128: column sums carry ghostmax's +1
                            nc.vector.tensor_scalar(acc[:, 0:QP], eab[:, 0:QP],
                                                    1.0 / 128.0, None, ALU.add)
                            nc.gpsimd.tensor_scalar(acc[:, QP:2 * QP],
                                                    eab[:, QP:2 * QP],
                                                    1.0 / 128.0, None, ALU.add)
                        else:
                            nc.vector.tensor_tensor(acc[:, 0:QP], acc[:, 0:QP],
                                                    eab[:, 0:QP], ALU.add)
                            b_eng = nc.gpsimd if c % 5 < 2 else nc.vector
                            b_eng.tensor_tensor(acc[:, QP:2 * QP],
                                                acc[:, QP:2 * QP],
                                                eab[:, QP:2 * QP], ALU.add)
                        mm(o0, vts[h][c][:], eab[:, 0:QP], c == 0,
                           c == NCH - 1)
                        mm(o1, vts[h][c][:], eab[:, QP:2 * QP], c == 0,
                           c == NCH - 1)
                        if c == 1:
                            for f in pending:
                                f()
                            pending = []
                    pending.append(
                        make_epilogue(h, qp, qsl, acc, o0, o1, oct_t, stats))
                    if qp == NPASS - 1:
                        pending.append(finish_head(h, oct_t, stats))
            for f in pending:
                f()
            pending = []

            # ---- deferred GroupNorm apply (one ln/exp table switch) ----
            lnv = work.tile([1, VH], F32, tag="lnv")
            nc.scalar.activation(lnv[:], vars2[:], AF.Ln)
            invs = work.tile([1, VH], F32, tag="invs")
            nc.scalar.activation(invs[:], lnv[:], AF.Exp, scale=-0.5)
            for h in range(VH):
                inv02 = work.tile([1, 1], F32, tag="inv02")
                nc.vector.tensor_scalar(inv02[:], invs[:, h:h + 1],
                                        1.0 - LAMBDA_INIT, None, ALU.mult)
                # broadcast inv02 and mean to [128,1] via hi/lo PE matmuls
                bco = work.tile([1, 4], BF16, tag="bco")
                blo = work.tile([1, 2], F32, tag="blo")
                nc.vector.tensor_copy(bco[:, 0:1], inv02[:])
                nc.vector.tensor_tensor(blo[:, 0:1], inv02[:], bco[:, 0:1],
                                        ALU.subtract)
                nc.vector.tensor_copy(bco[:, 1:2], blo[:, 0:1])
                nc.vector.tensor_copy(bco[:, 2:3], means2[:, h:h + 1])
                nc.vector.tensor_tensor(blo[:, 1:2], means2[:, h:h + 1],
                                        bco[:, 2:3], ALU.subtract)
                nc.vector.tensor_copy(bco[:, 3:4], blo[:, 1:2])
                bc_ps = ps.tile([128, QP], F32, tag="pab")
                nc.tensor.matmul(bc_ps[:, 0:1], ones[0:1, :], bco[:, 0:1],
                                 start=True, stop=False)
                nc.tensor.matmul(bc_ps[:, 0:1], ones[0:1, :], bco[:, 1:2],
                                 start=False, stop=True)
                nc.tensor.matmul(bc_ps[:, 1:2], ones[0:1, :], bco[:, 2:3],
                                 start=True, stop=False)
                nc.tensor.matmul(bc_ps[:, 1:2], ones[0:1, :], bco[:, 3:4],
                                 start=False, stop=True)
                inv02v = work.tile([128, 1], F32, tag="inv02v")
                negmv = work.tile([128, 1], F32, tag="negmv")
                nc.vector.tensor_copy(inv02v[:], bc_ps[:, 0:1])
                nc.vector.tensor_scalar(negmv[:], bc_ps[:, 1:2], -1.0, None,
                                        ALU.mult)

                # A[p,tt] = wq*inv*0.2 ; B[p,tt] = A*(-mean) + bq*0.2
                a16 = work.tile([128, NCH], F32, tag="a16")
                b16 = work.tile([128, NCH], F32, tag="b16")
                nc.vector.tensor_scalar(a16[:], wqs[h][:], inv02v[:], None,
                                        ALU.mult)
                nc.vector.scalar_tensor_tensor(
                    b16[:], a16[:], negmv[:], bqs[h][:], ALU.mult, ALU.add)
                outf = work.tile([128, S], F32, tag="outf")
                for tt in range(NCH):
                    tsl = slice(tt * 128, (tt + 1) * 128)
                    nc.vector.tensor_scalar(outf[:, tsl], octs[h][:, tsl],
                                            a16[:, tt:tt + 1],
                                            b16[:, tt:tt + 1],
                                            ALU.mult, ALU.add)
                nc.sync.dma_start(out_d[h], outf[:])

    nc.finalize()
    return nc


def _get_program():
    global _PROGRAM
    if _PROGRAM is None:
        _PROGRAM = _build_program()
    return _PROGRAM


def _prepare_in_maps(q, k, v, lambda_q1, lambda_k1, lambda_q2, lambda_k2,
                     gn_weight, gn_bias):
    q = np.asarray(q)
    k = np.asarray(k)
    v = np.asarray(v)

    lam = np.concatenate([np.asarray(lambda_q1), np.asarray(lambda_k1),
                          np.asarray(lambda_q2), np.asarray(lambda_k2)]
                         ).astype(np.float32).reshape(1, 4 * D)
    # gn params: channel c = h*128 + s//16 -> value per (head, query s)
    w_hq = np.asarray(gn_weight, dtype=np.float32).reshape(HQ, 128)
    b_hq = np.asarray(gn_bias, dtype=np.float32).reshape(HQ, 128)
    w_q = np.repeat(w_hq, 16, axis=1)                    # [HQ, 2048]
    b_q = np.repeat(b_hq, 16, axis=1) * (1.0 - LAMBDA_INIT)
    # device layout [128, 16]: entry [p, tt] = w_q[h, tt*128 + p]
    w_t = w_q.reshape(HQ, NCH, 128).transpose(0, 2, 1).copy()
    b_t = b_q.reshape(HQ, NCH, 128).transpose(0, 2, 1).copy()

    in_maps = []
    for core in range(NCORE):
        heads = [core * VH + i for i in range(VH)]
        qt = np.empty((VH, D, 2 * S), dtype=ml_dtypes.bfloat16)
        kt = np.empty((VH, D, 2 * S), dtype=ml_dtypes.bfloat16)
        vv = np.empty((VH, S, DV), dtype=ml_dtypes.bfloat16)
        wq16 = np.empty((VH, 128, NCH), dtype=np.float32)
        bq16 = np.empty((VH, 128, NCH), dtype=np.float32)
        for i, hh in enumerate(heads):
            qt[i, :, 0:S] = q[0, 2 * hh].T.astype(ml_dtypes.bfloat16)
            qt[i, :, S:2 * S] = q[0, 2 * hh + 1].T.astype(ml_dtypes.bfloat16)
            kt[i, :, 0:S] = k[0, 2 * hh].T.astype(ml_dtypes.bfloat16)
            kt[i, :, S:2 * S] = k[0, 2 * hh + 1].T.astype(ml_dtypes.bfloat16)
            vv[i] = v[0, hh].astype(ml_dtypes.bfloat16)
            wq16[i] = w_t[hh]
            bq16[i] = b_t[hh]
        in_maps.append({"qt": qt, "kt": kt, "v": vv, "lam": lam,
                        "wq": wq16, "bq": bq16})
    return in_maps


def _assemble(results):
    # out[vh] layout: [128 p, 16 tt, 128 d] -> head output [s=tt*128+p, d]
    out_heads = np.empty((HQ, S, DV), dtype=np.float32)
    for core in range(NCORE):
        o = results[core]["out"]                         # [VH, 128, 2048]
        for i in range(VH):
            oh = np.asarray(o[i]).reshape(128, NCH, DV)
            out_heads[core * VH + i] = oh.transpose(1, 0, 2).reshape(S, DV)
    x = out_heads.reshape(HQ * DV, S)                    # [C, S] row-major
    return np.ascontiguousarray(x.T)[None]               # [1, S, C]


def kernel(**inputs):
    nc = _get_program()
    in_maps = _prepare_in_maps(**inputs)
    res = run_bass_kernel_spmd(nc, in_maps, list(range(NCORE)))
    return _assemble(res.results)



# revision 6
# speedup vs baseline: 1.1686x; 1.1686x over previous
"""Differential-attention + GroupNorm Trainium2 kernel, 8-core head-parallel.

Problem (hardcoded):
  q, k: [1, 32, 2048, 64] f32 ; v: [1, 16, 2048, 128] f32
  lambda_q1/k1/q2/k2: [64] f32 ; gn_weight/gn_bias: [2048] f32
  out:  [1, 2048, 2048] f32

Sharding: 2 v-heads (= 4 q/k heads) per core across 8 cores.

Per core, per v-head, per 512-query pass: scores for both difference maps in
a keys-on-partitions layout ([128 k-chunk, 512 q0 | 512 q1] PSUM), exp on the
scalar engine (the binding resource: 128 x ~1.04us activations), ghostmax
denominators accumulated elementwise on DVE (fp16) and column-reduced with one
gpsimd partition_all_reduce, unnormalized AV accumulated in PSUM. The combine
(o0/d0 - lambda*o1/d1) happens post-AV in the [dv, q] layout with per-column
reciprocal rows, so no PE transposes or PSUM round trips are needed; GroupNorm
statistics ride along as fused accum outputs and the normalization is applied
per head in [dv, q] with broadcast-AP coefficient rows. The host transposes
each head's [dv, q] tile when assembling (pure reshape/cast, no device work).

PE queue is software-pipelined (scores of chunk c+1 are emitted before AV of
chunk c) so the exp stream never waits on the chunk chain.

Device inputs per core:
  qt  [2, 128, 2048] bf16 : rows 0:64 = q[2h]^T, rows 64:128 = q[2h+1]^T
  kt  [2, 128, 2048] bf16 : same for k
  vt  [2, 128, 2048] bf16 : vt[p, 128c+j] = v[h, 128c+p, j]
  lam [1, 256]       f32  : lambda_q1 | lambda_k1 | lambda_q2 | lambda_k2
  wq  [1, 256]       f32  : gn_weight per (head, s//16)
  bq  [1, 256]       f32  : gn_bias * (1-LAMBDA_INIT), same layout
Output:
  out [2, 128, 2048] bf16 : per head, out[h][d, s] (channels x positions)
"""
import math
import numpy as np
import ml_dtypes

import concourse.bass as bass
import concourse.mybir as mybir
import concourse.tile as tile
import concourse.bass_isa as bass_isa
from concourse import bacc
from concourse.bass_utils import run_bass_kernel_spmd

F32 = mybir.dt.float32
FP16 = mybir.dt.float16
BF16 = mybir.dt.bfloat16
AF = mybir.ActivationFunctionType
ALU = mybir.AluOpType
RED = bass_isa.ReduceOp

S = 2048          # sequence length
D = 64            # head dim of q/k
DV = 128          # head dim of v
HQ = 16           # number of v-heads
NCORE = 8
VH = HQ // NCORE  # v-heads per core = 2
QP = 512          # queries per pass
NPASS = S // QP   # 4
NCH = S // 128    # 16 key chunks
LAMBDA_INIT = 0.8
EPS = 1e-5
SCALE = 1.0 / math.sqrt(D)

_PROGRAM = None


def _build_program():
    nc = bacc.Bacc("TRN2", target_bir_lowering=False, debug=False,
                   num_devices=NCORE)
    qt_d = nc.dram_tensor("qt", [VH, 128, S], BF16, kind="ExternalInput").ap()
    kt_d = nc.dram_tensor("kt", [VH, 128, S], BF16, kind="ExternalInput").ap()
    vt_d = nc.dram_tensor("vt", [VH, 128, S], BF16, kind="ExternalInput").ap()
    lam_d = nc.dram_tensor("lam", [1, 4 * D], F32, kind="ExternalInput").ap()
    wq_d = nc.dram_tensor("wq", [1, VH * 128], F32, kind="ExternalInput").ap()
    bq_d = nc.dram_tensor("bq", [1, VH * 128], F32, kind="ExternalInput").ap()
    out_d = nc.dram_tensor("out", [VH, 128, S], BF16, kind="ExternalOutput").ap()

    inv_n = 1.0 / float(S * DV)

    with tile.TileContext(nc) as tc:
        with tc.tile_pool(name="const", bufs=1) as const, \
             tc.tile_pool(name="inp", bufs=1) as inp, \
             tc.tile_pool(name="eabp", bufs=4) as eabp, \
             tc.tile_pool(name="accp", bufs=2) as accp, \
             tc.tile_pool(name="dp", bufs=2) as dp, \
             tc.tile_pool(name="work", bufs=2) as work, \
             tc.tile_pool(name="sm", bufs=1) as sm, \
             tc.tile_pool(name="ps", bufs=2, space="PSUM") as ps, \
             tc.tile_pool(name="pso", bufs=2, space="PSUM") as pso:

            # ---- PE p-state warm-up (independent of inputs) ----
            wsc = const.tile([128, 512], BF16, tag="wsc")
            nc.gpsimd.memset(wsc[:], 0.5)
            for _w in range(6):
                wps = ps.tile([128, 1024], F32, tag="pab")
                nc.tensor.matmul(wps[:, 0:512], wsc[:, 0:128], wsc[:],
                                 start=True, stop=True)

            # ---- inputs (priority order: head 0 first) ----
            qts, kts, vts = [], [], []
            for h in range(VH):
                kt = inp.tile([128, S], BF16, tag=f"kt{h}")
                qt = inp.tile([128, S], BF16, tag=f"qt{h}")
                vt = inp.tile([128, S], BF16, tag=f"vt{h}")
                nc.sync.dma_start(kt[:], kt_d[h])
                nc.sync.dma_start(qt[:], qt_d[h])
                nc.sync.dma_start(vt[:], vt_d[h])
                qts.append(qt)
                kts.append(kt)
                vts.append(vt)
            lam = inp.tile([1, 4 * D], F32, tag="lam")
            nc.sync.dma_start(lam[:], lam_d[:])
            wqr = inp.tile([1, VH * 128], F32, tag="wqr")
            bqr = inp.tile([1, VH * 128], F32, tag="bqr")
            nc.sync.dma_start(wqr[:], wq_d[:])
            nc.sync.dma_start(bqr[:], bq_d[:])

            # ---- lambda_full = exp(lq1.lk1) - exp(lq2.lk2) + 0.8 ----
            scr = sm.tile([1, D], F32, tag="lscr")
            s12 = sm.tile([1, 2], F32, tag="ls12")
            nc.vector.tensor_tensor(scr[:], lam[:, 0:D], lam[:, D:2 * D],
                                    ALU.mult)
            nc.vector.tensor_reduce(s12[:, 0:1], scr[:],
                                    mybir.AxisListType.X, ALU.add)
            nc.vector.tensor_tensor(scr[:], lam[:, 2 * D:3 * D],
                                    lam[:, 3 * D:4 * D], ALU.mult)
            nc.vector.tensor_reduce(s12[:, 1:2], scr[:],
                                    mybir.AxisListType.X, ALU.add)
            e12 = sm.tile([1, 2], F32, tag="le12")
            nc.scalar.activation(e12[:], s12[:], AF.Exp)
            lamf = sm.tile([1, 1], F32, tag="lamf")
            nc.vector.tensor_tensor(lamf[:], e12[:, 0:1], e12[:, 1:2],
                                    ALU.subtract)
            nc.vector.tensor_scalar(lamf[:], lamf[:], LAMBDA_INIT, None,
                                    ALU.add)
            rlamf = sm.tile([1, 1], F32, tag="rlamf")
            nc.vector.reciprocal(rlamf[:], lamf[:])
            invlamv = const.tile([128, 1], F32, tag="invlamv")
            nc.gpsimd.partition_broadcast(invlamv[:], rlamf[:])

            # ---- GroupNorm row params broadcast across partitions ----
            wqb, bqb = [], []
            for h in range(VH):
                wb = const.tile([128, 128], F32, tag=f"wqb{h}")
                bb = const.tile([128, 128], F32, tag=f"bqb{h}")
                nc.gpsimd.partition_broadcast(wb[:], wqr[:, h * 128:(h + 1) * 128])
                nc.gpsimd.partition_broadcast(bb[:], bqr[:, h * 128:(h + 1) * 128])
                wqb.append(wb)
                bqb.append(bb)

            octs, sums, sqs = [], [], []
            for h in range(VH):
                oct_t = inp.tile([128, S], BF16, tag=f"oct{h}")
                sums_t = inp.tile([128, NPASS], F32, tag=f"sums{h}")
                sqs_t = inp.tile([128, NPASS], F32, tag=f"sqs{h}")
                octs.append(oct_t)
                sums.append(sums_t)
                sqs.append(sqs_t)

            def make_epilogue(h, qp, acc, o01):
                qsl = slice(qp * QP, (qp + 1) * QP)

                def epi():
                    dt = dp.tile([128, 2 * QP], F32, tag="dt")
                    nc.gpsimd.partition_all_reduce(dt[:], acc[:], 128, RED.add)
                    # r1' = lambda/d1: scale d1 by 1/lambda before recip
                    nc.vector.tensor_scalar(dt[:, QP:2 * QP], dt[:, QP:2 * QP],
                                            invlamv[:], None, ALU.mult)
                    rt = dp.tile([128, 2 * QP], F32, tag="rt")
                    nc.vector.reciprocal(rt[:], dt[:])
                    t0 = work.tile([128, QP], F32, tag="t0")
                    t1 = work.tile([128, QP], F32, tag="t1")
                    nc.vector.tensor_tensor(t1[:], o01[:, QP:2 * QP],
                                            rt[:, QP:2 * QP], ALU.mult)
                    nc.vector.tensor_tensor(t0[:], o01[:, 0:QP],
                                            rt[:, 0:QP], ALU.mult)
                    nc.vector.scalar_tensor_tensor(
                        octs[h][:, qsl], t0[:], 1.0, t1[:],
                        ALU.mult, ALU.subtract,
                        accum_out=sums[h][:, qp:qp + 1])
                    scr2 = work.tile([128, QP], BF16, tag="scr2")
                    nc.vector.scalar_tensor_tensor(
                        scr2[:], octs[h][:, qsl], 1.0, octs[h][:, qsl],
                        ALU.mult, ALU.mult,
                        accum_out=sqs[h][:, qp:qp + 1])
                return epi

            def make_gn(h):
                def gn():
                    st = sm.tile([128, 4], F32, tag="st")
                    nc.vector.tensor_reduce(st[:, 0:1], sums[h][:],
                                            mybir.AxisListType.X, ALU.add)
                    nc.vector.tensor_reduce(st[:, 1:2], sqs[h][:],
                                            mybir.AxisListType.X, ALU.add)
                    nc.gpsimd.partition_all_reduce(st[:, 0:2], st[:, 0:2],
                                                   128, RED.add)
                    mu = sm.tile([128, 4], F32, tag="mu")
                    # mu, E[x^2], var+eps, -mu
                    nc.vector.tensor_scalar(mu[:, 0:1], st[:, 0:1], inv_n,
                                            None, ALU.mult)
                    nc.vector.tensor_scalar(mu[:, 1:2], st[:, 1:2], inv_n,
                                            None, ALU.mult)
                    nc.vector.scalar_tensor_tensor(
                        mu[:, 2:3], mu[:, 0:1], 1.0, mu[:, 0:1],
                        ALU.mult, ALU.mult)                 # mu^2
                    nc.vector.tensor_tensor(mu[:, 2:3], mu[:, 1:2],
                                            mu[:, 2:3], ALU.subtract)  # var
                    nc.vector.tensor_scalar(mu[:, 2:3], mu[:, 2:3], EPS,
                                            None, ALU.add)
                    nc.vector.tensor_scalar(mu[:, 3:4], mu[:, 0:1], -1.0,
                                            None, ALU.mult)            # -mu
                    # 1/sigma = exp(-0.5 * ln(var)), same ACT table as Exp
                    iv = sm.tile([128, 2], F32, tag="iv")
                    nc.scalar.activation(iv[:, 0:1], mu[:, 2:3], AF.Ln)
                    nc.scalar.activation(iv[:, 1:2], iv[:, 0:1], AF.Exp,
                                         scale=-0.5)
                    i02 = sm.tile([128, 1], F32, tag="i02")
                    nc.vector.tensor_scalar(i02[:], iv[:, 1:2],
                                            1.0 - LAMBDA_INIT, None, ALU.mult)
                    # A = w * i02 ; B = b*0.2 + (-mu) * A
                    a16 = sm.tile([128, 128], F32, tag="a16")
                    b16 = sm.tile([128, 128], F32, tag="b16")
                    nc.vector.tensor_scalar(a16[:], wqb[h][:], i02[:],
                                            None, ALU.mult)
                    nc.vector.scalar_tensor_tensor(
                        b16[:], a16[:], mu[:, 3:4], bqb[h][:],
                        ALU.mult, ALU.add)
                    # out = oct * A[q>>4] + B[q>>4]
                    outf = inp.tile([128, S], BF16, tag=f"outf{h}")
                    tmp = inp.tile([128, S], F32, tag=f"gtmp{h}")
                    nc.vector.tensor_tensor(
                        tmp[:].rearrange("p (c s) -> p c s", c=128),
                        octs[h][:].rearrange("p (c s) -> p c s", c=128),
                        a16[:].rearrange("p (c one) -> p c one", one=1)
                            .broadcast_to([128, 128, 16]),
                        ALU.mult)
                    nc.vector.tensor_tensor(
                        outf[:].rearrange("p (c s) -> p c s", c=128),
                        tmp[:].rearrange("p (c s) -> p c s", c=128),
                        b16[:].rearrange("p (c one) -> p c one", one=1)
                            .broadcast_to([128, 128, 16]),
                        ALU.add)
                    nc.sync.dma_start(out_d[h], outf[:])
                return gn

            # ---- main pipeline ----
            pending = []
            for h in range(VH):
                for qp in range(NPASS):
                    qsl = slice(qp * QP, (qp + 1) * QP)
                    acc = accp.tile([128, 2 * QP], FP16, tag="acc")
                    o01 = pso.tile([128, 2 * QP], F32, tag="o01")
                    eabs = []
                    for c in range(NCH):
                        csl = slice(c * 128, (c + 1) * 128)
                        pab = ps.tile([128, 2 * QP], F32, tag="pab")
                        nc.tensor.matmul(pab[:, 0:QP], kts[h][0:64, csl],
                                         qts[h][0:64, qsl],
                                         start=True, stop=True)
                        nc.tensor.matmul(pab[:, QP:2 * QP], kts[h][64:128, csl],
                                         qts[h][64:128, qsl],
                                         start=True, stop=True)
                        eab = eabp.tile([128, 2 * QP], BF16, tag="eab")
                        nc.scalar.activation(eab[:], pab[:], AF.Exp,
                                             scale=SCALE)
                        eabs.append(eab)
                        if c == 0:
                            # ghostmax: +1/128 per partition carries the +1
                            nc.vector.tensor_scalar(acc[:], eab[:],
                                                    1.0 / 128.0, None, ALU.add)
                        else:
                            nc.vector.tensor_tensor(acc[:], acc[:], eab[:],
                                                    ALU.add)
                        if c >= 1:
                            pcsl = slice((c - 1) * 128, c * 128)
                            pe = eabs[c - 1]
                            nc.tensor.matmul(o01[:, 0:QP], vts[h][:, pcsl],
                                             pe[:, 0:QP],
                                             start=(c == 1), stop=False)
                            nc.tensor.matmul(o01[:, QP:2 * QP], vts[h][:, pcsl],
                                             pe[:, QP:2 * QP],
                                             start=(c == 1), stop=False)
                        if c == 3 and pending:
                            for f in pending:
                                f()
                            pending = []
                    lcsl = slice((NCH - 1) * 128, NCH * 128)
                    nc.tensor.matmul(o01[:, 0:QP], vts[h][:, lcsl],
                                     eabs[NCH - 1][:, 0:QP],
                                     start=False, stop=True)
                    nc.tensor.matmul(o01[:, QP:2 * QP], vts[h][:, lcsl],
                                     eabs[NCH - 1][:, QP:2 * QP],
                                     start=False, stop=True)
                    pending.append(make_epilogue(h, qp, acc, o01))
                    if qp == NPASS - 1:
                        pending.append(make_gn(h))
            for f in pending:
                f()

    nc.finalize()
    return nc


def _get_program():
    global _PROGRAM
    if _PROGRAM is None:
        _PROGRAM = _build_program()
    return _PROGRAM


def _prepare_in_maps(q, k, v, lambda_q1, lambda_k1, lambda_q2, lambda_k2,
                     gn_weight, gn_bias):
    q = np.asarray(q)
    k = np.asarray(k)
    v = np.asarray(v)

    lam = np.concatenate([np.asarray(lambda_q1), np.asarray(lambda_k1),
                          np.asarray(lambda_q2), np.asarray(lambda_k2)]
                         ).astype(np.float32).reshape(1, 4 * D)
    w_hq = np.asarray(gn_weight, dtype=np.float32).reshape(HQ, 128)
    b_hq = np.asarray(gn_bias, dtype=np.float32).reshape(HQ, 128) \
        * (1.0 - LAMBDA_INIT)

    in_maps = []
    for core in range(NCORE):
        heads = [core * VH + i for i in range(VH)]
        qt = np.empty((VH, 128, S), dtype=ml_dtypes.bfloat16)
        kt = np.empty((VH, 128, S), dtype=ml_dtypes.bfloat16)
        vt = np.empty((VH, 128, S), dtype=ml_dtypes.bfloat16)
        wq = np.empty((1, VH * 128), dtype=np.float32)
        bq = np.empty((1, VH * 128), dtype=np.float32)
        for i, hh in enumerate(heads):
            qt[i, 0:64] = q[0, 2 * hh].T.astype(ml_dtypes.bfloat16)
            qt[i, 64:128] = q[0, 2 * hh + 1].T.astype(ml_dtypes.bfloat16)
            kt[i, 0:64] = k[0, 2 * hh].T.astype(ml_dtypes.bfloat16)
            kt[i, 64:128] = k[0, 2 * hh + 1].T.astype(ml_dtypes.bfloat16)
            vt[i] = (v[0, hh].reshape(NCH, 128, DV).transpose(1, 0, 2)
                     .reshape(128, S).astype(ml_dtypes.bfloat16))
            wq[0, i * 128:(i + 1) * 128] = w_hq[hh]
            bq[0, i * 128:(i + 1) * 128] = b_hq[hh]
        in_maps.append({"qt": qt, "kt": kt, "vt": vt, "lam": lam,
                        "wq": wq, "bq": bq})
    return in_maps


def _assemble(results):
    # device out[h] = [dv, s]; head output is [s, dv]
    out_heads = np.empty((HQ, S, DV), dtype=np.float32)
    for core in range(NCORE):
        o = results[core]["out"]                      # [VH, 128, 2048] bf16
        for i in range(VH):
            out_heads[core * VH + i] = np.asarray(o[i]).astype(np.float32).T
    x = out_heads.reshape(HQ * DV, S)                 # torch-style flatten
    return np.ascontiguousarray(x.T)[None]            # [1, S, C]


def kernel(**inputs):
    nc = _get_program()
    in_maps = _prepare_in_maps(**inputs)
    res = run_bass_kernel_spmd(nc, in_maps, list(range(NCORE)))
    return _assemble(res.results)


# revision 8
# speedup vs baseline: 1.2643x; 1.0819x over previous
"""Differential-attention + GroupNorm Trainium2 kernel, 8-core head-parallel.

Problem (hardcoded):
  q, k: [1, 32, 2048, 64] f32 ; v: [1, 16, 2048, 128] f32
  lambda_q1/k1/q2/k2: [64] f32 ; gn_weight/gn_bias: [2048] f32
  out:  [1, 2048, 2048] f32

Sharding: 2 v-heads (= 4 q/k heads) per core across 8 cores.

Per core the work is a flat stream of 128 key-chunk steps (2 heads x 4
query-passes x 16 chunks). Scores for both difference maps land in a
keys-on-partitions PSUM tile ([128 k, 512 q0 | 512 q1]); exp on the scalar
engine is the binding resource (128 x ~1.04us activations), so the PE queue
is software-pipelined with the AV matmuls lagging the scores by two steps —
the exp stream never waits on the chunk chain. Ghostmax denominators are
accumulated elementwise on DVE (fp16, +1/128 seed carries the ghost logit)
and column-reduced with one gpsimd partition_all_reduce per pass. The
combine (o0/d0 - lambda*o1/d1) happens post-AV in the [dv, q] layout with
per-column reciprocal rows (no PE transposes, no PSUM round trips);
GroupNorm statistics ride along as fused accum outputs, 1/sigma comes from
a magic-constant Newton rsqrt on DVE (no activation-table switch), and the
normalization is applied per head in [dv, q] with broadcast-AP coefficient
rows. The host transposes each head's [dv, q] tile when assembling.

Device inputs per core:
  qt  [2, 128, 2048] bf16 : rows 0:64 = q[2h]^T, rows 64:128 = q[2h+1]^T
  kt  [2, 128, 2048] bf16 : same for k
  vt  [2, 128, 2048] bf16 : vt[p, 128c+j] = v[h, 128c+p, j]
  lam [1, 256]       f32  : lambda_q1 | lambda_k1 | lambda_q2 | lambda_k2
  wq  [1, 256]       f32  : gn_weight per (head, s//16)
  bq  [1, 256]       f32  : gn_bias * (1-LAMBDA_INIT), same layout
Output:
  out [2, 128, 2048] bf16 : per head, out[h][d, s] (channels x positions)
"""
import math
import numpy as np
import ml_dtypes

import concourse.bass as bass
import concourse.mybir as mybir
import concourse.tile as tile
import concourse.bass_isa as bass_isa
from concourse import bacc
from concourse.bass_utils import run_bass_kernel_spmd

F32 = mybir.dt.float32
FP16 = mybir.dt.float16
BF16 = mybir.dt.bfloat16
I32 = mybir.dt.int32
AF = mybir.ActivationFunctionType
ALU = mybir.AluOpType
RED = bass_isa.ReduceOp

S = 2048          # sequence length
D = 64            # head dim of q/k
DV = 128          # head dim of v
HQ = 16           # number of v-heads
NCORE = 8
VH = HQ // NCORE  # v-heads per core = 2
QP = 512          # queries per pass
NPASS = S // QP   # 4
NCH = S // 128    # 16 key chunks
NP = VH * NPASS   # 8 passes
G = NP * NCH      # 128 global chunk steps
LAMBDA_INIT = 0.8
EPS = 1e-5
SCALE = 1.0 / math.sqrt(D)
MAGIC = 0x5F3759DF

_PROGRAM = None


def _build_program():
    nc = bacc.Bacc("TRN2", target_bir_lowering=False, debug=False,
                   num_devices=NCORE)
    qt_d = nc.dram_tensor("qt", [VH, 128, S], BF16, kind="ExternalInput").ap()
    kt_d = nc.dram_tensor("kt", [VH, 128, S], BF16, kind="ExternalInput").ap()
    vt_d = nc.dram_tensor("vt", [VH, 128, S], BF16, kind="ExternalInput").ap()
    lam_d = nc.dram_tensor("lam", [1, 4 * D], F32, kind="ExternalInput").ap()
    wq_d = nc.dram_tensor("wq", [1, VH * 128], F32, kind="ExternalInput").ap()
    bq_d = nc.dram_tensor("bq", [1, VH * 128], F32, kind="ExternalInput").ap()
    out_d = nc.dram_tensor("out", [VH, 128, S], BF16, kind="ExternalOutput").ap()

    inv_n = 1.0 / float(S * DV)

    with tile.TileContext(nc) as tc:
        with tc.tile_pool(name="const", bufs=1) as const, \
             tc.tile_pool(name="inp", bufs=1) as inp, \
             tc.tile_pool(name="eabp", bufs=4) as eabp, \
             tc.tile_pool(name="accp", bufs=2) as accp, \
             tc.tile_pool(name="dp", bufs=2) as dp, \
             tc.tile_pool(name="work", bufs=2) as work, \
             tc.tile_pool(name="sm", bufs=1) as sm, \
             tc.tile_pool(name="ps", bufs=2, space="PSUM") as ps, \
             tc.tile_pool(name="pso", bufs=2, space="PSUM") as pso:

            # ---- PE p-state warm-up (independent of inputs) ----
            wsc = const.tile([128, 512], BF16, tag="wsc")
            nc.gpsimd.memset(wsc[:], 0.5)
            for _w in range(6):
                wps = ps.tile([128, 1024], F32, tag="pab")
                nc.tensor.matmul(wps[:, 0:512], wsc[:, 0:128], wsc[:],
                                 start=True, stop=True)

            # ---- inputs; head 0 first, qt on a second DGE queue ----
            qts, kts, vts = [], [], []
            for h in range(VH):
                kt = inp.tile([128, S], BF16, tag=f"kt{h}")
                qt = inp.tile([128, S], BF16, tag=f"qt{h}")
                vt = inp.tile([128, S], BF16, tag=f"vt{h}")
                nc.sync.dma_start(kt[:], kt_d[h])
                nc.scalar.dma_start(qt[:], qt_d[h])
                nc.sync.dma_start(vt[:], vt_d[h])
                qts.append(qt)
                kts.append(kt)
                vts.append(vt)
            lam = inp.tile([1, 4 * D], F32, tag="lam")
            wqr = inp.tile([1, VH * 128], F32, tag="wqr")
            bqr = inp.tile([1, VH * 128], F32, tag="bqr")
            nc.sync.dma_start(lam[:], lam_d[:])
            nc.sync.dma_start(wqr[:], wq_d[:])
            nc.sync.dma_start(bqr[:], bq_d[:])

            invlamv = const.tile([128, 1], F32, tag="invlamv")
            wqb, bqb = [], []
            for h in range(VH):
                wb = const.tile([128, 128], F32, tag=f"wqb{h}")
                bb = const.tile([128, 128], F32, tag=f"bqb{h}")
                wqb.append(wb)
                bqb.append(bb)

            def make_prep():
                def prep():
                    # lambda_full = exp(lq1.lk1) - exp(lq2.lk2) + 0.8
                    scr = sm.tile([1, D], F32, tag="lscr")
                    s12 = sm.tile([1, 2], F32, tag="ls12")
                    nc.vector.tensor_tensor(scr[:], lam[:, 0:D],
                                            lam[:, D:2 * D], ALU.mult)
                    nc.vector.tensor_reduce(s12[:, 0:1], scr[:],
                                            mybir.AxisListType.X, ALU.add)
                    nc.vector.tensor_tensor(scr[:], lam[:, 2 * D:3 * D],
                                            lam[:, 3 * D:4 * D], ALU.mult)
                    nc.vector.tensor_reduce(s12[:, 1:2], scr[:],
                                            mybir.AxisListType.X, ALU.add)
                    e12 = sm.tile([1, 2], F32, tag="le12")
                    nc.scalar.activation(e12[:], s12[:], AF.Exp)
                    lamf = sm.tile([1, 1], F32, tag="lamf")
                    nc.vector.tensor_tensor(lamf[:], e12[:, 0:1], e12[:, 1:2],
                                            ALU.subtract)
                    nc.vector.tensor_scalar(lamf[:], lamf[:], LAMBDA_INIT,
                                            None, ALU.add)
                    rlamf = sm.tile([1, 1], F32, tag="rlamf")
                    nc.vector.reciprocal(rlamf[:], lamf[:])
                    nc.gpsimd.partition_broadcast(invlamv[:], rlamf[:])
                    for h in range(VH):
                        nc.gpsimd.partition_broadcast(
                            wqb[h][:], wqr[:, h * 128:(h + 1) * 128])
                        nc.gpsimd.partition_broadcast(
                            bqb[h][:], bqr[:, h * 128:(h + 1) * 128])
                return prep

            octs, sums, sqs = [], [], []
            for h in range(VH):
                oct_t = inp.tile([128, S], BF16, tag=f"oct{h}")
                sums_t = inp.tile([128, NPASS], F32, tag=f"sums{h}")
                sqs_t = inp.tile([128, NPASS], F32, tag=f"sqs{h}")
                octs.append(oct_t)
                sums.append(sums_t)
                sqs.append(sqs_t)

            def make_epilogue(h, qp, acc, o01):
                qsl = slice(qp * QP, (qp + 1) * QP)

                def epi():
                    dt = dp.tile([128, 2 * QP], F32, tag="dt")
                    nc.gpsimd.partition_all_reduce(dt[:], acc[:], 128, RED.add)
                    rt = dp.tile([128, 2 * QP], F32, tag="rt")
                    t0 = work.tile([128, QP], F32, tag="t0")
                    t1 = work.tile([128, QP], F32, tag="t1")
                    nc.vector.reciprocal(rt[:, 0:QP], dt[:, 0:QP])
                    nc.vector.tensor_tensor(t0[:], o01[:, 0:QP],
                                            rt[:, 0:QP], ALU.mult)
                    # r1' = lambda/d1: scale d1 by 1/lambda before recip
                    nc.vector.tensor_scalar(dt[:, QP:2 * QP], dt[:, QP:2 * QP],
                                            invlamv[:], None, ALU.mult)
                    nc.vector.reciprocal(rt[:, QP:2 * QP], dt[:, QP:2 * QP])
                    nc.vector.tensor_tensor(t1[:], o01[:, QP:2 * QP],
                                            rt[:, QP:2 * QP], ALU.mult)
                    nc.vector.scalar_tensor_tensor(
                        octs[h][:, qsl], t0[:], 1.0, t1[:],
                        ALU.mult, ALU.subtract,
                        accum_out=sums[h][:, qp:qp + 1])
                    scr2 = work.tile([128, QP], BF16, tag="scr2")
                    nc.vector.scalar_tensor_tensor(
                        scr2[:], octs[h][:, qsl], 1.0, octs[h][:, qsl],
                        ALU.mult, ALU.mult,
                        accum_out=sqs[h][:, qp:qp + 1])
                return epi

            def make_gn(h):
                def gn():
                    st = sm.tile([128, 4], F32, tag="st")
                    nc.vector.tensor_reduce(st[:, 0:1], sums[h][:],
                                            mybir.AxisListType.X, ALU.add)
                    nc.vector.tensor_reduce(st[:, 1:2], sqs[h][:],
                                            mybir.AxisListType.X, ALU.add)
                    nc.gpsimd.partition_all_reduce(st[:, 0:2], st[:, 0:2],
                                                   128, RED.add)
                    mu = sm.tile([128, 4], F32, tag="mu")
                    nc.vector.tensor_scalar(mu[:, 0:1], st[:, 0:1], inv_n,
                                            None, ALU.mult)
                    nc.vector.tensor_scalar(mu[:, 1:2], st[:, 1:2], inv_n,
                                            None, ALU.mult)
                    nc.vector.scalar_tensor_tensor(
                        mu[:, 2:3], mu[:, 0:1], 1.0, mu[:, 0:1],
                        ALU.mult, ALU.mult)                 # mu^2
                    nc.vector.tensor_tensor(mu[:, 2:3], mu[:, 1:2],
                                            mu[:, 2:3], ALU.subtract)  # var
                    nc.vector.tensor_scalar(mu[:, 2:3], mu[:, 2:3], EPS,
                                            None, ALU.add)
                    nc.vector.tensor_scalar(mu[:, 3:4], mu[:, 0:1], -1.0,
                                            None, ALU.mult)            # -mu
                    # 1/sigma: magic-constant Newton rsqrt, all on DVE
                    sh = sm.tile([128, 1], I32, tag="sh")
                    nc.vector.tensor_scalar(sh[:], mu[:, 2:3].bitcast(I32),
                                            1, None, ALU.logical_shift_right)
                    nc.vector.tensor_scalar(sh[:], sh[:], -1, None, ALU.mult)
                    nc.vector.tensor_scalar(sh[:], sh[:], MAGIC, None, ALU.add)
                    y = sm.tile([128, 1], F32, tag="y")
                    nc.vector.tensor_copy(y[:].bitcast(I32), sh[:])
                    t = sm.tile([128, 1], F32, tag="t")
                    for _ in range(3):
                        nc.vector.tensor_tensor(t[:], y[:], y[:], ALU.mult)
                        nc.vector.tensor_tensor(t[:], t[:], mu[:, 2:3],
                                                ALU.mult)
                        nc.vector.tensor_scalar(t[:], t[:], -0.5, 1.5,
                                                ALU.mult, ALU.add)
                        nc.vector.tensor_tensor(y[:], y[:], t[:], ALU.mult)
                    i02 = sm.tile([128, 1], F32, tag="i02")
                    nc.vector.tensor_scalar(i02[:], y[:],
                                            1.0 - LAMBDA_INIT, None, ALU.mult)
                    # A = w * i02 ; B = b*0.2 + (-mu) * A
                    a16 = sm.tile([128, 128], F32, tag="a16")
                    b16 = sm.tile([128, 128], F32, tag="b16")
                    nc.vector.tensor_scalar(a16[:], wqb[h][:], i02[:],
                                            None, ALU.mult)
                    nc.vector.scalar_tensor_tensor(
                        b16[:], a16[:], mu[:, 3:4], bqb[h][:],
                        ALU.mult, ALU.add)
                    # out = oct * A[q>>4] + B[q>>4], split in halves so the
                    # first DMA overlaps the second half's apply
                    outf = inp.tile([128, S], BF16, tag=f"outf{h}")
                    tmp = inp.tile([128, S], F32, tag=f"gtmp{h}")
                    for half in range(2):
                        hs = slice(half * (S // 2), (half + 1) * (S // 2))
                        ha = slice(half * 64, (half + 1) * 64)
                        nc.vector.tensor_tensor(
                            tmp[:, hs].rearrange("p (c s) -> p c s", c=64),
                            octs[h][:, hs].rearrange("p (c s) -> p c s", c=64),
                            a16[:, ha].rearrange("p (c one) -> p c one", one=1)
                                .broadcast_to([128, 64, 16]),
                            ALU.mult)
                        nc.vector.tensor_tensor(
                            outf[:, hs].rearrange("p (c s) -> p c s", c=64),
                            tmp[:, hs].rearrange("p (c s) -> p c s", c=64),
                            b16[:, ha].rearrange("p (c one) -> p c one", one=1)
                                .broadcast_to([128, 64, 16]),
                            ALU.add)
                        nc.sync.dma_start(out_d[h, :, hs], outf[:, hs])
                return gn

            # ---- main pipeline: flat over 128 global chunk steps ----
            passes = [(h, qp) for h in range(VH) for qp in range(NPASS)]
            accs = {}
            o01s = {}
            eabs = {}
            pending = [make_prep()]

            def emit_scores_exp_acc(g):
                p, c = g // NCH, g % NCH
                h, qp = passes[p]
                qsl = slice(qp * QP, (qp + 1) * QP)
                csl = slice(c * 128, (c + 1) * 128)
                if c == 0:
                    acc = accp.tile([128, 2 * QP], FP16, tag="acc")
                    o01 = pso.tile([128, 2 * QP], F32, tag="o01")
                    accs[p] = acc
                    o01s[p] = o01
                pab = ps.tile([128, 2 * QP], F32, tag="pab")
                nc.tensor.matmul(pab[:, 0:QP], kts[h][0:64, csl],
                                 qts[h][0:64, qsl], start=True, stop=True)
                nc.tensor.matmul(pab[:, QP:2 * QP], kts[h][64:128, csl],
                                 qts[h][64:128, qsl], start=True, stop=True)
                eab = eabp.tile([128, 2 * QP], BF16, tag="eab")
                nc.scalar.activation(eab[:], pab[:], AF.Exp, scale=SCALE)
                eabs[g] = eab
                acc = accs[p]
                if c == 0:
                    # ghostmax: +1/128 per partition carries the +1
                    nc.vector.tensor_scalar(acc[:], eab[:], 1.0 / 128.0,
                                            None, ALU.add)
                else:
                    nc.vector.tensor_tensor(acc[:], acc[:], eab[:], ALU.add)

            def emit_av(g):
                p, c = g // NCH, g % NCH
                h, _ = passes[p]
                csl = slice(c * 128, (c + 1) * 128)
                o01 = o01s[p]
                eab = eabs.pop(g)
                nc.tensor.matmul(o01[:, 0:QP], vts[h][:, csl], eab[:, 0:QP],
                                 start=(c == 0), stop=(c == NCH - 1))
                nc.tensor.matmul(o01[:, QP:2 * QP], vts[h][:, csl],
                                 eab[:, QP:2 * QP],
                                 start=(c == 0), stop=(c == NCH - 1))

            for g in range(G + 2):
                if g < G:
                    emit_scores_exp_acc(g)
                if g >= 2:
                    emit_av(g - 2)
                    gp = g - 2
                    if gp % NCH == NCH - 1:
                        p = gp // NCH
                        h, qp = passes[p]
                        pending.append(
                            make_epilogue(h, qp, accs.pop(p), o01s.pop(p)))
                        if qp == NPASS - 1:
                            pending.append(make_gn(h))
                if g % NCH == 6 and pending:
                    for f in pending:
                        f()
                    pending = []
            for f in pending:
                f()

    nc.finalize()
    return nc


def _get_program():
    global _PROGRAM
    if _PROGRAM is None:
        _PROGRAM = _build_program()
    return _PROGRAM


def _prepare_in_maps(q, k, v, lambda_q1, lambda_k1, lambda_q2, lambda_k2,
                     gn_weight, gn_bias):
    q = np.asarray(q)
    k = np.asarray(k)
    v = np.asarray(v)

    lam = np.concatenate([np.asarray(lambda_q1), np.asarray(lambda_k1),
                          np.asarray(lambda_q2), np.asarray(lambda_k2)]
                         ).astype(np.float32).reshape(1, 4 * D)
    w_hq = np.asarray(gn_weight, dtype=np.float32).reshape(HQ, 128)
    b_hq = np.asarray(gn_bias, dtype=np.float32).reshape(HQ, 128) \
        * (1.0 - LAMBDA_INIT)

    in_maps = []
    for core in range(NCORE):
        heads = [core * VH + i for i in range(VH)]
        qt = np.empty((VH, 128, S), dtype=ml_dtypes.bfloat16)
        kt = np.empty((VH, 128, S), dtype=ml_dtypes.bfloat16)
        vt = np.empty((VH, 128, S), dtype=ml_dtypes.bfloat16)
        wq = np.empty((1, VH * 128), dtype=np.float32)
        bq = np.empty((1, VH * 128), dtype=np.float32)
        for i, hh in enumerate(heads):
            qt[i, 0:64] = q[0, 2 * hh].T.astype(ml_dtypes.bfloat16)
            qt[i, 64:128] = q[0, 2 * hh + 1].T.astype(ml_dtypes.bfloat16)
            kt[i, 0:64] = k[0, 2 * hh].T.astype(ml_dtypes.bfloat16)
            kt[i, 64:128] = k[0, 2 * hh + 1].T.astype(ml_dtypes.bfloat16)
            vt[i] = (v[0, hh].reshape(NCH, 128, DV).transpose(1, 0, 2)
                     .reshape(128, S).astype(ml_dtypes.bfloat16))
            wq[0, i * 128:(i + 1) * 128] = w_hq[hh]
            bq[0, i * 128:(i + 1) * 128] = b_hq[hh]
        in_maps.append({"qt": qt, "kt": kt, "vt": vt, "lam": lam,
                        "wq": wq, "bq": bq})
    return in_maps


def _assemble(results):
    # device out[h] = [dv, s]; head output is [s, dv]
    out_heads = np.empty((HQ, S, DV), dtype=np.float32)
    for core in range(NCORE):
        o = results[core]["out"]                      # [VH, 128, 2048] bf16
        for i in range(VH):
            out_heads[core * VH + i] = np.asarray(o[i]).astype(np.float32).T
    x = out_heads.reshape(HQ * DV, S)                 # torch-style flatten
    return np.ascontiguousarray(x.T)[None]            # [1, S, C]


def kernel(**inputs):
    nc = _get_program()
    in_maps = _prepare_in_maps(**inputs)
    res = run_bass_kernel_spmd(nc, in_maps, list(range(NCORE)))
    return _assemble(res.results)


# revision 23
# speedup vs baseline: 1.2922x; 1.0220x over previous
"""Differential-attention + GroupNorm Trainium2 kernel, 8-core head-parallel.

Problem (hardcoded):
  q, k: [1, 32, 2048, 64] f32 ; v: [1, 16, 2048, 128] f32
  lambda_q1/k1/q2/k2: [64] f32 ; gn_weight/gn_bias: [2048] f32
  out:  [1, 2048, 2048] f32

Sharding: 2 v-heads (= 4 q/k heads) per core across 8 cores.

Per core the work is a flat stream of 128 key-chunk steps (2 heads x 4
query-passes x 16 chunks). Scores for both difference maps land in a
keys-on-partitions PSUM tile ([128 k, 512 q0 | 512 q1]); exp on the scalar
engine is the binding resource (128 x ~1.04us activations), so the PE queue
is software-pipelined with the AV matmuls lagging the scores by two steps —
the exp stream never waits on the chunk chain. Ghostmax denominators are
accumulated elementwise on DVE (fp16, +1/128 seed carries the ghost logit)
and column-reduced with one gpsimd partition_all_reduce per pass. The
combine (o0/d0 - lambda*o1/d1) happens post-AV in the [dv, q] layout with
per-column reciprocal rows (no PE transposes, no PSUM round trips);
GroupNorm statistics ride along as fused accum outputs, 1/sigma comes from
a magic-constant Newton rsqrt on DVE (no activation-table switch), and the
normalization is applied per head in [dv, q] with broadcast-AP coefficient
rows. The host transposes each head's [dv, q] tile when assembling.

Device inputs per core:
  qt  [2, 128, 2048] bf16 : rows 0:64 = q[2h]^T, rows 64:128 = q[2h+1]^T
  kt  [2, 128, 2048] bf16 : same for k
  vt  [2, 128, 2048] bf16 : vt[p, 128c+j] = v[h, 128c+p, j]
  lam [1, 256]       f32  : lambda_q1 | lambda_k1 | lambda_q2 | lambda_k2
  wq  [1, 256]       f32  : gn_weight per (head, s//16)
  bq  [1, 256]       f32  : gn_bias * (1-LAMBDA_INIT), same layout
Output:
  out [2, 128, 2048] bf16 : per head, out[h][d, s] (channels x positions)
"""
import math
import numpy as np
import ml_dtypes

import concourse.bass as bass
import concourse.mybir as mybir
import concourse.tile as tile
import concourse.bass_isa as bass_isa
from concourse import bacc
from concourse.bass_utils import run_bass_kernel_spmd

F32 = mybir.dt.float32
FP16 = mybir.dt.float16
BF16 = mybir.dt.bfloat16
I32 = mybir.dt.int32
AF = mybir.ActivationFunctionType
ALU = mybir.AluOpType
RED = bass_isa.ReduceOp

S = 2048          # sequence length
D = 64            # head dim of q/k
DV = 128          # head dim of v
HQ = 16           # number of v-heads
NCORE = 8
VH = HQ // NCORE  # v-heads per core = 2
QP = 512          # queries per pass
NPASS = S // QP   # 4
NCH = S // 128    # 16 key chunks
NP = VH * NPASS   # 8 passes
G = NP * NCH      # 128 global chunk steps
LAMBDA_INIT = 0.8
EPS = 1e-5
SCALE = 1.0 / math.sqrt(D)
MAGIC = 0x5F3759DF

_PROGRAM = None


def _build_program():
    nc = bacc.Bacc("TRN2", target_bir_lowering=False, debug=False,
                   num_devices=NCORE)
    qt_d = nc.dram_tensor("qt", [VH, 128, S], BF16, kind="ExternalInput").ap()
    kt_d = nc.dram_tensor("kt", [VH, 128, S], BF16, kind="ExternalInput").ap()
    vt_d = nc.dram_tensor("vt", [VH, 128, S], BF16, kind="ExternalInput").ap()
    lam_d = nc.dram_tensor("lam", [1, 4 * D], F32, kind="ExternalInput").ap()
    wq_d = nc.dram_tensor("wq", [1, VH * 128], F32, kind="ExternalInput").ap()
    bq_d = nc.dram_tensor("bq", [1, VH * 128], F32, kind="ExternalInput").ap()
    out_d = nc.dram_tensor("out", [VH, 128, S], BF16, kind="ExternalOutput").ap()

    inv_n = 1.0 / float(S * DV)

    with tile.TileContext(nc) as tc:
        with tc.tile_pool(name="const", bufs=1) as const, \
             tc.tile_pool(name="inp", bufs=1) as inp, \
             tc.tile_pool(name="eabp", bufs=6) as eabp, \
             tc.tile_pool(name="accp", bufs=2) as accp, \
             tc.tile_pool(name="dp", bufs=2) as dp, \
             tc.tile_pool(name="work", bufs=2) as work, \
             tc.tile_pool(name="sm", bufs=1) as sm, \
             tc.tile_pool(name="ps", bufs=2, space="PSUM") as ps, \
             tc.tile_pool(name="pso", bufs=2, space="PSUM") as pso:

            # ---- PE p-state warm-up (independent of inputs) ----
            wsc = const.tile([128, 512], BF16, tag="wsc")
            nc.gpsimd.memset(wsc[:], 0.5)
            for _w in range(6):
                wps = ps.tile([128, 1024], F32, tag="pab")
                nc.tensor.matmul(wps[:, 0:512], wsc[:, 0:128], wsc[:],
                                 start=True, stop=True)

            # ---- inputs; head 0 split fine-grained so chunk 0 starts early,
            # qt on the Activation DGE queue to parallelize the ramp ----
            qts, kts, vts = [], [], []
            for h in range(VH):
                kt = inp.tile([128, S], BF16, tag=f"kt{h}")
                qt = inp.tile([128, S], BF16, tag=f"qt{h}")
                vt = inp.tile([128, S], BF16, tag=f"vt{h}")
                qts.append(qt)
                kts.append(kt)
                vts.append(vt)
            lam = inp.tile([1, 4 * D], F32, tag="lam")
            wqr = inp.tile([1, VH * 128], F32, tag="wqr")
            bqr = inp.tile([1, VH * 128], F32, tag="bqr")
            # SP queue: head-0 k/v first (split so chunk 0 starts early),
            # then the small param rows, then head 1.
            nc.sync.dma_start(kts[0][:, 0:512], kt_d[0][:, 0:512])
            nc.sync.dma_start(vts[0][:, 0:512], vt_d[0][:, 0:512])
            nc.sync.dma_start(wqr[:], wq_d[:])
            nc.sync.dma_start(bqr[:], bq_d[:])
            nc.sync.dma_start(kts[0][:, 512:S], kt_d[0][:, 512:S])
            nc.sync.dma_start(vts[0][:, 512:S], vt_d[0][:, 512:S])
            nc.sync.dma_start(kts[1][:], kt_d[1])
            nc.sync.dma_start(vts[1][:], vt_d[1])
            # Activation queue (parallel): q and lambda
            nc.scalar.dma_start(qts[0][:, 0:512], qt_d[0][:, 0:512])
            nc.scalar.dma_start(lam[:], lam_d[:])
            nc.scalar.dma_start(qts[0][:, 512:S], qt_d[0][:, 512:S])
            nc.scalar.dma_start(qts[1][:], qt_d[1])

            invlamv = const.tile([128, 1], F32, tag="invlamv")
            wqb, bqb = [], []
            for h in range(VH):
                wb = const.tile([128, 128], F32, tag=f"wqb{h}")
                bb = const.tile([128, 128], F32, tag=f"bqb{h}")
                wqb.append(wb)
                bqb.append(bb)

            def make_prep():
                def prep():
                    # lambda_full = exp(lq1.lk1) - exp(lq2.lk2) + 0.8
                    scr = sm.tile([1, D], F32, tag="lscr")
                    s12 = sm.tile([1, 2], F32, tag="ls12")
                    nc.vector.tensor_tensor(scr[:], lam[:, 0:D],
                                            lam[:, D:2 * D], ALU.mult)
                    nc.vector.tensor_reduce(s12[:, 0:1], scr[:],
                                            mybir.AxisListType.X, ALU.add)
                    nc.vector.tensor_tensor(scr[:], lam[:, 2 * D:3 * D],
                                            lam[:, 3 * D:4 * D], ALU.mult)
                    nc.vector.tensor_reduce(s12[:, 1:2], scr[:],
                                            mybir.AxisListType.X, ALU.add)
                    e12 = sm.tile([1, 2], F32, tag="le12")
                    nc.scalar.activation(e12[:], s12[:], AF.Exp)
                    lamf = sm.tile([1, 1], F32, tag="lamf")
                    nc.vector.tensor_tensor(lamf[:], e12[:, 0:1], e12[:, 1:2],
                                            ALU.subtract)
                    nc.vector.tensor_scalar(lamf[:], lamf[:], LAMBDA_INIT,
                                            None, ALU.add)
                    rlamf = sm.tile([1, 1], F32, tag="rlamf")
                    nc.vector.reciprocal(rlamf[:], lamf[:])
                    nc.gpsimd.partition_broadcast(invlamv[:], rlamf[:])
                    for h in range(VH):
                        nc.gpsimd.partition_broadcast(
                            wqb[h][:], wqr[:, h * 128:(h + 1) * 128])
                        nc.gpsimd.partition_broadcast(
                            bqb[h][:], bqr[:, h * 128:(h + 1) * 128])
                return prep

            octs, sums, sqs = [], [], []
            for h in range(VH):
                oct_t = inp.tile([128, S], BF16, tag=f"oct{h}")
                sums_t = inp.tile([128, NPASS], F32, tag=f"sums{h}")
                sqs_t = inp.tile([128, NPASS], F32, tag=f"sqs{h}")
                octs.append(oct_t)
                sums.append(sums_t)
                sqs.append(sqs_t)

            def make_epilogue(h, qp, acc, o01):
                qsl = slice(qp * QP, (qp + 1) * QP)

                dts = []

                def epi_a():
                    # column sums per map half on Pool (d0 first, so its
                    # reciprocal can overlap d1's all_reduce); fold 1/lambda
                    # into acc's d1 half so r1 = lambda/d1
                    dt = dp.tile([128, 2 * QP], F32, tag="dt")
                    rt = dp.tile([128, 2 * QP], F32, tag="rt")
                    dts.append((dt, rt))
                    nc.gpsimd.partition_all_reduce(dt[:, 0:QP], acc[:, 0:QP],
                                                   128, RED.add)
                    nc.vector.tensor_scalar(acc[:, QP:2 * QP],
                                            acc[:, QP:2 * QP],
                                            invlamv[:], None, ALU.mult)
                    nc.gpsimd.partition_all_reduce(dt[:, QP:2 * QP],
                                                   acc[:, QP:2 * QP],
                                                   128, RED.add)

                def epi_b():
                    dt, rt = dts[0]
                    t0 = work.tile([128, QP], F32, tag="t0")
                    nc.vector.reciprocal(rt[:, 0:QP], dt[:, 0:QP])
                    nc.vector.tensor_tensor(t0[:], o01[:, 0:QP],
                                            rt[:, 0:QP], ALU.mult)
                    dts.append(t0)

                def epi_c():
                    dt, rt = dts[0]
                    t0 = dts[1]
                    t1 = work.tile([128, QP], F32, tag="t1")
                    nc.vector.reciprocal(rt[:, QP:2 * QP], dt[:, QP:2 * QP])
                    nc.vector.tensor_tensor(t1[:], o01[:, QP:2 * QP],
                                            rt[:, QP:2 * QP], ALU.mult)
                    nc.vector.scalar_tensor_tensor(
                        octs[h][:, qsl], t0[:], 1.0, t1[:],
                        ALU.mult, ALU.subtract,
                        accum_out=sums[h][:, qp:qp + 1])

                def epi_d():
                    scr2 = work.tile([128, QP], BF16, tag="scr2")
                    nc.vector.scalar_tensor_tensor(
                        scr2[:], octs[h][:, qsl], 1.0, octs[h][:, qsl],
                        ALU.mult, ALU.mult,
                        accum_out=sqs[h][:, qp:qp + 1])

                return [epi_a, epi_b, epi_c, epi_d]

            def make_gn(h):
                def gn_stats():
                    st = sm.tile([128, 4], F32, tag="st")
                    nc.vector.tensor_reduce(st[:, 0:1], sums[h][:],
                                            mybir.AxisListType.X, ALU.add)
                    nc.vector.tensor_reduce(st[:, 1:2], sqs[h][:],
                                            mybir.AxisListType.X, ALU.add)
                    nc.gpsimd.partition_all_reduce(st[:, 0:2], st[:, 0:2],
                                                   128, RED.add)
                    mu = sm.tile([128, 4], F32, tag="mu")
                    nc.vector.tensor_scalar(mu[:, 0:2], st[:, 0:2], inv_n,
                                            None, ALU.mult)  # mu | E[x^2]
                    nc.vector.scalar_tensor_tensor(
                        mu[:, 2:3], mu[:, 0:1], 1.0, mu[:, 0:1],
                        ALU.mult, ALU.mult)                 # mu^2
                    nc.vector.tensor_tensor(mu[:, 2:3], mu[:, 1:2],
                                            mu[:, 2:3], ALU.subtract)  # var
                    nc.vector.tensor_scalar(mu[:, 2:3], mu[:, 2:3], EPS,
                                            None, ALU.add)
                    nc.vector.tensor_scalar(mu[:, 3:4], mu[:, 0:1], -1.0,
                                            None, ALU.mult)            # -mu
                    # 1/sigma: magic-constant rsqrt + 2 Newton steps on DVE
                    sh = sm.tile([128, 1], I32, tag="sh")
                    nc.vector.tensor_scalar(sh[:], mu[:, 2:3].bitcast(I32),
                                            1, None, ALU.logical_shift_right)
                    nc.vector.tensor_scalar(sh[:], sh[:], -1, MAGIC,
                                            ALU.mult, ALU.add)
                    y = sm.tile([128, 1], F32, tag="y")
                    nc.vector.tensor_copy(y[:].bitcast(I32), sh[:])
                    t = sm.tile([128, 1], F32, tag="t")
                    hna = sm.tile([128, 1], F32, tag="hna")
                    nc.vector.tensor_scalar(hna[:], mu[:, 2:3], -0.5, None,
                                            ALU.mult)
                    for _ in range(2):
                        nc.vector.tensor_tensor(t[:], y[:], y[:], ALU.mult)
                        nc.vector.tensor_scalar(t[:], t[:], hna[:], 1.5,
                                                ALU.mult, ALU.add)
                        nc.vector.tensor_tensor(y[:], y[:], t[:], ALU.mult)
                    i02 = sm.tile([128, 1], F32, tag="i02")
                    nc.vector.tensor_scalar(i02[:], y[:],
                                            1.0 - LAMBDA_INIT, None, ALU.mult)
                    # A = w * i02 ; B = b*0.2 + (-mu) * A
                    a16 = sm.tile([128, 128], F32, tag=f"a16{h}")
                    b16 = sm.tile([128, 128], F32, tag=f"b16{h}")
                    nc.vector.tensor_scalar(a16[:], wqb[h][:], i02[:],
                                            None, ALU.mult)
                    nc.vector.scalar_tensor_tensor(
                        b16[:], a16[:], mu[:, 3:4], bqb[h][:],
                        ALU.mult, ALU.add)
                    return a16, b16

                coeffs = []
                pieces = [lambda: coeffs.append(gn_stats())]

                # out = oct * A[q>>4] + B[q>>4], quartered so DMAs overlap
                # the remaining applies and no DVE op clogs the FIFO
                outf = inp.tile([128, S], BF16, tag=f"outf{h}")
                tmp = inp.tile([128, S], F32, tag=f"gtmp{h}")

                def make_quarter(qu):
                    def quarter():
                        a16, b16 = coeffs[0]
                        qs = slice(qu * (S // 4), (qu + 1) * (S // 4))
                        qa = slice(qu * 32, (qu + 1) * 32)
                        nc.vector.tensor_tensor(
                            tmp[:, qs].rearrange("p (c s) -> p c s", c=32),
                            octs[h][:, qs].rearrange("p (c s) -> p c s", c=32),
                            a16[:, qa].rearrange("p (c one) -> p c one", one=1)
                                .broadcast_to([128, 32, 16]),
                            ALU.mult)
                        nc.vector.tensor_tensor(
                            outf[:, qs].rearrange("p (c s) -> p c s", c=32),
                            tmp[:, qs].rearrange("p (c s) -> p c s", c=32),
                            b16[:, qa].rearrange("p (c one) -> p c one", one=1)
                                .broadcast_to([128, 32, 16]),
                            ALU.add)
                        nc.sync.dma_start(out_d[h, :, qs], outf[:, qs])
                    return quarter

                for qu in range(4):
                    pieces.append(make_quarter(qu))
                return pieces

            # ---- main pipeline: flat over 128 global chunk steps ----
            passes = [(h, qp) for h in range(VH) for qp in range(NPASS)]
            accs = {}
            o01s = {}
            eabs = {}
            pending = [make_prep()]  # queue of small emitters, 1 per step

            def emit_scores_exp_acc(g):
                p, c = g // NCH, g % NCH
                h, qp = passes[p]
                qsl = slice(qp * QP, (qp + 1) * QP)
                csl = slice(c * 128, (c + 1) * 128)
                if c == 0:
                    acc = accp.tile([128, 2 * QP], FP16, tag="acc")
                    o01 = pso.tile([128, 2 * QP], F32, tag="o01")
                    accs[p] = acc
                    o01s[p] = o01
                pab = ps.tile([128, 2 * QP], F32, tag="pab")
                nc.tensor.matmul(pab[:, 0:QP], kts[h][0:64, csl],
                                 qts[h][0:64, qsl], start=True, stop=True)
                nc.tensor.matmul(pab[:, QP:2 * QP], kts[h][64:128, csl],
                                 qts[h][64:128, qsl], start=True, stop=True)
                eab = eabp.tile([128, 2 * QP], BF16, tag="eab")
                nc.scalar.activation(eab[:], pab[:], AF.Exp, scale=SCALE)
                eabs[g] = eab
                acc = accs[p]
                if c == 0:
                    # ghostmax: +1/128 per partition carries the +1
                    nc.vector.tensor_scalar(acc[:], eab[:], 1.0 / 128.0,
                                            None, ALU.add)
                else:
                    nc.vector.tensor_tensor(acc[:], acc[:], eab[:], ALU.add)

            def emit_av(g):
                p, c = g // NCH, g % NCH
                h, _ = passes[p]
                csl = slice(c * 128, (c + 1) * 128)
                o01 = o01s[p]
                eab = eabs.pop(g)
                nc.tensor.matmul(o01[:, 0:QP], vts[h][:, csl], eab[:, 0:QP],
                                 start=(c == 0), stop=(c == NCH - 1))
                nc.tensor.matmul(o01[:, QP:2 * QP], vts[h][:, csl],
                                 eab[:, QP:2 * QP],
                                 start=(c == 0), stop=(c == NCH - 1))

            for g in range(G + 2):
                if g < G:
                    emit_scores_exp_acc(g)
                if g >= 2:
                    emit_av(g - 2)
                    gp = g - 2
                    if gp % NCH == NCH - 1:
                        p = gp // NCH
                        h, qp = passes[p]
                        pending.extend(
                            make_epilogue(h, qp, accs.pop(p), o01s.pop(p)))
                        if qp == NPASS - 1:
                            pending.extend(make_gn(h))
                    if pending:
                        pending.pop(0)()
            for f in pending:
                f()

    nc.finalize()
    return nc


def _get_program():
    global _PROGRAM
    if _PROGRAM is None:
        _PROGRAM = _build_program()
    return _PROGRAM


def _prepare_in_maps(q, k, v, lambda_q1, lambda_k1, lambda_q2, lambda_k2,
                     gn_weight, gn_bias):
    q = np.asarray(q)
    k = np.asarray(k)
    v = np.asarray(v)

    lam = np.concatenate([np.asarray(lambda_q1), np.asarray(lambda_k1),
                          np.asarray(lambda_q2), np.asarray(lambda_k2)]
                         ).astype(np.float32).reshape(1, 4 * D)
    w_hq = np.asarray(gn_weight, dtype=np.float32).reshape(HQ, 128)
    b_hq = np.asarray(gn_bias, dtype=np.float32).reshape(HQ, 128) \
        * (1.0 - LAMBDA_INIT)

    in_maps = []
    for core in range(NCORE):
        heads = [core * VH + i for i in range(VH)]
        qt = np.empty((VH, 128, S), dtype=ml_dtypes.bfloat16)
        kt = np.empty((VH, 128, S), dtype=ml_dtypes.bfloat16)
        vt = np.empty((VH, 128, S), dtype=ml_dtypes.bfloat16)
        wq = np.empty((1, VH * 128), dtype=np.float32)
        bq = np.empty((1, VH * 128), dtype=np.float32)
        for i, hh in enumerate(heads):
            qt[i, 0:64] = q[0, 2 * hh].T.astype(ml_dtypes.bfloat16)
            qt[i, 64:128] = q[0, 2 * hh + 1].T.astype(ml_dtypes.bfloat16)
            kt[i, 0:64] = k[0, 2 * hh].T.astype(ml_dtypes.bfloat16)
            kt[i, 64:128] = k[0, 2 * hh + 1].T.astype(ml_dtypes.bfloat16)
            vt[i] = (v[0, hh].reshape(NCH, 128, DV).transpose(1, 0, 2)
                     .reshape(128, S).astype(ml_dtypes.bfloat16))
            wq[0, i * 128:(i + 1) * 128] = w_hq[hh]
            bq[0, i * 128:(i + 1) * 128] = b_hq[hh]
        in_maps.append({"qt": qt, "kt": kt, "vt": vt, "lam": lam,
                        "wq": wq, "bq": bq})
    return in_maps


def _assemble(results):
    # device out[h] = [dv, s]; head output is [s, dv]
    out_heads = np.empty((HQ, S, DV), dtype=np.float32)
    for core in range(NCORE):
        o = results[core]["out"]                      # [VH, 128, 2048] bf16
        for i in range(VH):
            out_heads[core * VH + i] = np.asarray(o[i]).astype(np.float32).T
    x = out_heads.reshape(HQ * DV, S)                 # torch-style flatten
    return np.ascontiguousarray(x.T)[None]            # [1, S, C]


def kernel(**inputs):
    nc = _get_program()
    in_maps = _prepare_in_maps(**inputs)
    res = run_bass_kernel_spmd(nc, in_maps, list(range(NCORE)))
    return _assemble(res.results)


# revision 25
# speedup vs baseline: 1.2964x; 1.0033x over previous
"""Differential-attention + GroupNorm Trainium2 kernel, 8-core head-parallel.

Problem (hardcoded):
  q, k: [1, 32, 2048, 64] f32 ; v: [1, 16, 2048, 128] f32
  lambda_q1/k1/q2/k2: [64] f32 ; gn_weight/gn_bias: [2048] f32
  out:  [1, 2048, 2048] f32

Sharding: 2 v-heads (= 4 q/k heads) per core across 8 cores.

Per core the work is a flat stream of 128 key-chunk steps (2 heads x 4
query-passes x 16 chunks). Scores for both difference maps land in a
keys-on-partitions PSUM tile ([128 k, 512 q0 | 512 q1]); exp on the scalar
engine is the binding resource (128 x ~1.04us activations), so the PE queue
is software-pipelined with the AV matmuls lagging the scores by two steps —
the exp stream never waits on the chunk chain. Ghostmax denominators are
accumulated elementwise on DVE (fp16, +1/128 seed carries the ghost logit)
and column-reduced with one gpsimd partition_all_reduce per pass. The
combine (o0/d0 - lambda*o1/d1) happens post-AV in the [dv, q] layout with
per-column reciprocal rows (no PE transposes, no PSUM round trips);
GroupNorm statistics ride along as fused accum outputs, 1/sigma comes from
a magic-constant Newton rsqrt on DVE (no activation-table switch), and the
normalization is applied per head in [dv, q] with broadcast-AP coefficient
rows. The host transposes each head's [dv, q] tile when assembling.

Device inputs per core:
  qt  [2, 128, 2048] bf16 : rows 0:64 = q[2h]^T, rows 64:128 = q[2h+1]^T
  kt  [2, 128, 2048] bf16 : same for k
  vt  [2, 128, 2048] bf16 : vt[p, 128c+j] = v[h, 128c+p, j]
  lam [1, 256]       f32  : lambda_q1 | lambda_k1 | lambda_q2 | lambda_k2
  wq  [1, 256]       f32  : gn_weight per (head, s//16)
  bq  [1, 256]       f32  : gn_bias * (1-LAMBDA_INIT), same layout
Output:
  out [2, 128, 2048] bf16 : per head, out[h][d, s] (channels x positions)
"""
import math
import numpy as np
import ml_dtypes

import concourse.bass as bass
import concourse.mybir as mybir
import concourse.tile as tile
import concourse.bass_isa as bass_isa
from concourse import bacc
from concourse.bass_utils import run_bass_kernel_spmd

F32 = mybir.dt.float32
FP16 = mybir.dt.float16
BF16 = mybir.dt.bfloat16
I32 = mybir.dt.int32
AF = mybir.ActivationFunctionType
ALU = mybir.AluOpType
RED = bass_isa.ReduceOp

S = 2048          # sequence length
D = 64            # head dim of q/k
DV = 128          # head dim of v
HQ = 16           # number of v-heads
NCORE = 8
VH = HQ // NCORE  # v-heads per core = 2
QP = 512          # queries per pass
NPASS = S // QP   # 4
NCH = S // 128    # 16 key chunks
NP = VH * NPASS   # 8 passes
G = NP * NCH      # 128 global chunk steps
LAMBDA_INIT = 0.8
EPS = 1e-5
SCALE = 1.0 / math.sqrt(D)
MAGIC = 0x5F3759DF

_PROGRAM = None


def _build_program():
    nc = bacc.Bacc("TRN2", target_bir_lowering=False, debug=False,
                   num_devices=NCORE)
    qt_d = nc.dram_tensor("qt", [VH, 128, S], BF16, kind="ExternalInput").ap()
    kt_d = nc.dram_tensor("kt", [VH, 128, S], BF16, kind="ExternalInput").ap()
    vt_d = nc.dram_tensor("vt", [VH, 128, S], BF16, kind="ExternalInput").ap()
    lam_d = nc.dram_tensor("lam", [1, 4 * D], F32, kind="ExternalInput").ap()
    wq_d = nc.dram_tensor("wq", [1, VH * 128], F32, kind="ExternalInput").ap()
    bq_d = nc.dram_tensor("bq", [1, VH * 128], F32, kind="ExternalInput").ap()
    out_d = nc.dram_tensor("out", [VH, 128, S], BF16, kind="ExternalOutput").ap()

    inv_n = 1.0 / float(S * DV)

    with tile.TileContext(nc) as tc:
        with tc.tile_pool(name="const", bufs=1) as const, \
             tc.tile_pool(name="inp", bufs=1) as inp, \
             tc.tile_pool(name="eabp", bufs=8) as eabp, \
             tc.tile_pool(name="accp", bufs=2) as accp, \
             tc.tile_pool(name="dp", bufs=2) as dp, \
             tc.tile_pool(name="work", bufs=2) as work, \
             tc.tile_pool(name="sm", bufs=1) as sm, \
             tc.tile_pool(name="ps", bufs=2, space="PSUM") as ps, \
             tc.tile_pool(name="pso", bufs=2, space="PSUM") as pso:

            # ---- PE p-state warm-up (independent of inputs) ----
            wsc = const.tile([128, 512], BF16, tag="wsc")
            nc.gpsimd.memset(wsc[:], 0.5)
            for _w in range(6):
                wps = ps.tile([128, 1024], F32, tag="pab")
                nc.tensor.matmul(wps[:, 0:512], wsc[:, 0:128], wsc[:],
                                 start=True, stop=True)

            # ---- inputs; head 0 split fine-grained so chunk 0 starts early,
            # qt on the Activation DGE queue to parallelize the ramp ----
            qts, kts, vts = [], [], []
            for h in range(VH):
                kt = inp.tile([128, S], BF16, tag=f"kt{h}")
                qt = inp.tile([128, S], BF16, tag=f"qt{h}")
                vt = inp.tile([128, S], BF16, tag=f"vt{h}")
                qts.append(qt)
                kts.append(kt)
                vts.append(vt)
            lam = inp.tile([1, 4 * D], F32, tag="lam")
            wqr = inp.tile([1, VH * 128], F32, tag="wqr")
            bqr = inp.tile([1, VH * 128], F32, tag="bqr")
            # SP queue: head-0 k/v first (split so chunk 0 starts early),
            # then the small param rows, then head 1.
            nc.sync.dma_start(kts[0][:, 0:512], kt_d[0][:, 0:512])
            nc.sync.dma_start(vts[0][:, 0:512], vt_d[0][:, 0:512])
            nc.sync.dma_start(wqr[:], wq_d[:])
            nc.sync.dma_start(bqr[:], bq_d[:])
            nc.sync.dma_start(kts[0][:, 512:S], kt_d[0][:, 512:S])
            nc.sync.dma_start(vts[0][:, 512:S], vt_d[0][:, 512:S])
            nc.sync.dma_start(kts[1][:], kt_d[1])
            nc.sync.dma_start(vts[1][:], vt_d[1])
            # Activation queue (parallel): q and lambda
            nc.scalar.dma_start(qts[0][:, 0:512], qt_d[0][:, 0:512])
            nc.scalar.dma_start(lam[:], lam_d[:])
            nc.scalar.dma_start(qts[0][:, 512:S], qt_d[0][:, 512:S])
            nc.scalar.dma_start(qts[1][:], qt_d[1])

            invlamv = const.tile([128, 1], F32, tag="invlamv")
            wqb, bqb = [], []
            for h in range(VH):
                wb = const.tile([128, 128], F32, tag=f"wqb{h}")
                bb = const.tile([128, 128], F32, tag=f"bqb{h}")
                wqb.append(wb)
                bqb.append(bb)

            def make_prep():
                def prep():
                    # lambda_full = exp(lq1.lk1) - exp(lq2.lk2) + 0.8
                    scr = sm.tile([1, D], F32, tag="lscr")
                    s12 = sm.tile([1, 2], F32, tag="ls12")
                    nc.vector.tensor_tensor(scr[:], lam[:, 0:D],
                                            lam[:, D:2 * D], ALU.mult)
                    nc.vector.tensor_reduce(s12[:, 0:1], scr[:],
                                            mybir.AxisListType.X, ALU.add)
                    nc.vector.tensor_tensor(scr[:], lam[:, 2 * D:3 * D],
                                            lam[:, 3 * D:4 * D], ALU.mult)
                    nc.vector.tensor_reduce(s12[:, 1:2], scr[:],
                                            mybir.AxisListType.X, ALU.add)
                    e12 = sm.tile([1, 2], F32, tag="le12")
                    nc.scalar.activation(e12[:], s12[:], AF.Exp)
                    lamf = sm.tile([1, 1], F32, tag="lamf")
                    nc.vector.tensor_tensor(lamf[:], e12[:, 0:1], e12[:, 1:2],
                                            ALU.subtract)
                    nc.vector.tensor_scalar(lamf[:], lamf[:], LAMBDA_INIT,
                                            None, ALU.add)
                    rlamf = sm.tile([1, 1], F32, tag="rlamf")
                    nc.vector.reciprocal(rlamf[:], lamf[:])
                    nc.gpsimd.partition_broadcast(invlamv[:], rlamf[:])
                    for h in range(VH):
                        nc.gpsimd.partition_broadcast(
                            wqb[h][:], wqr[:, h * 128:(h + 1) * 128])
                        nc.gpsimd.partition_broadcast(
                            bqb[h][:], bqr[:, h * 128:(h + 1) * 128])
                return prep

            octs, sums, sqs = [], [], []
            for h in range(VH):
                oct_t = inp.tile([128, S], BF16, tag=f"oct{h}")
                sums_t = inp.tile([128, NPASS], F32, tag=f"sums{h}")
                sqs_t = inp.tile([128, NPASS], F32, tag=f"sqs{h}")
                octs.append(oct_t)
                sums.append(sums_t)
                sqs.append(sqs_t)

            def make_epilogue(h, qp, acc, o01):
                qsl = slice(qp * QP, (qp + 1) * QP)

                dts = []

                def epi_a():
                    # column sums per map half on Pool (d0 first, so its
                    # reciprocal can overlap d1's all_reduce); fold 1/lambda
                    # into acc's d1 half so r1 = lambda/d1
                    dt = dp.tile([128, 2 * QP], F32, tag="dt")
                    rt = dp.tile([128, 2 * QP], F32, tag="rt")
                    dts.append((dt, rt))
                    nc.gpsimd.partition_all_reduce(dt[:, 0:QP], acc[:, 0:QP],
                                                   128, RED.add)
                    nc.vector.tensor_scalar(acc[:, QP:2 * QP],
                                            acc[:, QP:2 * QP],
                                            invlamv[:], None, ALU.mult)
                    nc.gpsimd.partition_all_reduce(dt[:, QP:2 * QP],
                                                   acc[:, QP:2 * QP],
                                                   128, RED.add)

                def epi_b():
                    dt, rt = dts[0]
                    t0 = work.tile([128, QP], F32, tag="t0")
                    nc.vector.reciprocal(rt[:, 0:QP], dt[:, 0:QP])
                    nc.vector.tensor_tensor(t0[:], o01[:, 0:QP],
                                            rt[:, 0:QP], ALU.mult)
                    dts.append(t0)

                def epi_c():
                    dt, rt = dts[0]
                    t0 = dts[1]
                    t1 = work.tile([128, QP], F32, tag="t1")
                    nc.vector.reciprocal(rt[:, QP:2 * QP], dt[:, QP:2 * QP])
                    nc.vector.tensor_tensor(t1[:], o01[:, QP:2 * QP],
                                            rt[:, QP:2 * QP], ALU.mult)
                    nc.vector.scalar_tensor_tensor(
                        octs[h][:, qsl], t0[:], 1.0, t1[:],
                        ALU.mult, ALU.subtract,
                        accum_out=sums[h][:, qp:qp + 1])

                def epi_d():
                    scr2 = work.tile([128, QP], BF16, tag="scr2")
                    nc.vector.scalar_tensor_tensor(
                        scr2[:], octs[h][:, qsl], 1.0, octs[h][:, qsl],
                        ALU.mult, ALU.mult,
                        accum_out=sqs[h][:, qp:qp + 1])

                return [epi_a, epi_b, epi_c, epi_d]

            def make_gn(h):
                def gn_stats():
                    st = sm.tile([128, 4], F32, tag="st")
                    nc.vector.tensor_reduce(st[:, 0:1], sums[h][:],
                                            mybir.AxisListType.X, ALU.add)
                    nc.vector.tensor_reduce(st[:, 1:2], sqs[h][:],
                                            mybir.AxisListType.X, ALU.add)
                    nc.gpsimd.partition_all_reduce(st[:, 0:2], st[:, 0:2],
                                                   128, RED.add)
                    mu = sm.tile([128, 4], F32, tag="mu")
                    nc.vector.tensor_scalar(mu[:, 0:1], st[:, 0:1], inv_n,
                                            None, ALU.mult)            # mu
                    nc.vector.tensor_scalar(mu[:, 1:2], st[:, 1:2], inv_n,
                                            EPS, ALU.mult, ALU.add)  # E2+eps
                    nc.vector.tensor_scalar(mu[:, 3:4], mu[:, 0:1], -1.0,
                                            None, ALU.mult)            # -mu
                    nc.vector.scalar_tensor_tensor(
                        mu[:, 2:3], mu[:, 0:1], mu[:, 3:4], mu[:, 1:2],
                        ALU.mult, ALU.add)       # var+eps = E2+eps - mu^2
                    # 1/sigma: magic-constant rsqrt + 2 Newton steps on DVE
                    sh = sm.tile([128, 1], I32, tag="sh")
                    nc.vector.tensor_scalar(sh[:], mu[:, 2:3].bitcast(I32),
                                            1, None, ALU.logical_shift_right)
                    nc.vector.tensor_scalar(sh[:], sh[:], -1, MAGIC,
                                            ALU.mult, ALU.add)
                    y = sm.tile([128, 1], F32, tag="y")
                    nc.vector.tensor_copy(y[:].bitcast(I32), sh[:])
                    t = sm.tile([128, 1], F32, tag="t")
                    hna = sm.tile([128, 1], F32, tag="hna")
                    nc.vector.tensor_scalar(hna[:], mu[:, 2:3], -0.5, None,
                                            ALU.mult)
                    for _ in range(2):
                        nc.vector.tensor_tensor(t[:], y[:], y[:], ALU.mult)
                        nc.vector.tensor_scalar(t[:], t[:], hna[:], 1.5,
                                                ALU.mult, ALU.add)
                        nc.vector.tensor_tensor(y[:], y[:], t[:], ALU.mult)
                    i02 = sm.tile([128, 1], F32, tag="i02")
                    nc.vector.tensor_scalar(i02[:], y[:],
                                            1.0 - LAMBDA_INIT, None, ALU.mult)
                    # A = w * i02 ; B = b*0.2 + (-mu) * A
                    a16 = sm.tile([128, 128], F32, tag=f"a16{h}")
                    b16 = sm.tile([128, 128], F32, tag=f"b16{h}")
                    nc.vector.tensor_scalar(a16[:], wqb[h][:], i02[:],
                                            None, ALU.mult)
                    nc.vector.scalar_tensor_tensor(
                        b16[:], a16[:], mu[:, 3:4], bqb[h][:],
                        ALU.mult, ALU.add)
                    return a16, b16

                coeffs = []
                pieces = [lambda: coeffs.append(gn_stats())]

                # out = oct * A[q>>4] + B[q>>4], quartered so DMAs overlap
                # the remaining applies and no DVE op clogs the FIFO
                outf = inp.tile([128, S], BF16, tag=f"outf{h}")
                tmp = inp.tile([128, S], F32, tag=f"gtmp{h}")

                def make_quarter(qu):
                    def quarter():
                        a16, b16 = coeffs[0]
                        qs = slice(qu * (S // 4), (qu + 1) * (S // 4))
                        qa = slice(qu * 32, (qu + 1) * 32)
                        nc.vector.tensor_tensor(
                            tmp[:, qs].rearrange("p (c s) -> p c s", c=32),
                            octs[h][:, qs].rearrange("p (c s) -> p c s", c=32),
                            a16[:, qa].rearrange("p (c one) -> p c one", one=1)
                                .broadcast_to([128, 32, 16]),
                            ALU.mult)
                        nc.vector.tensor_tensor(
                            outf[:, qs].rearrange("p (c s) -> p c s", c=32),
                            tmp[:, qs].rearrange("p (c s) -> p c s", c=32),
                            b16[:, qa].rearrange("p (c one) -> p c one", one=1)
                                .broadcast_to([128, 32, 16]),
                            ALU.add)
                        nc.sync.dma_start(out_d[h, :, qs], outf[:, qs])
                    return quarter

                for qu in range(4):
                    pieces.append(make_quarter(qu))
                return pieces

            # ---- main pipeline: flat over 128 global chunk steps ----
            passes = [(h, qp) for h in range(VH) for qp in range(NPASS)]
            accs = {}
            o01s = {}
            eabs = {}
            pending = [make_prep()]  # queue of small emitters, 1 per step

            def emit_scores_exp_acc(g):
                p, c = g // NCH, g % NCH
                h, qp = passes[p]
                qsl = slice(qp * QP, (qp + 1) * QP)
                csl = slice(c * 128, (c + 1) * 128)
                if c == 0:
                    acc = accp.tile([128, 2 * QP], FP16, tag="acc")
                    o01 = pso.tile([128, 2 * QP], F32, tag="o01")
                    accs[p] = acc
                    o01s[p] = o01
                pab = ps.tile([128, 2 * QP], F32, tag="pab")
                nc.tensor.matmul(pab[:, 0:QP], kts[h][0:64, csl],
                                 qts[h][0:64, qsl], start=True, stop=True)
                nc.tensor.matmul(pab[:, QP:2 * QP], kts[h][64:128, csl],
                                 qts[h][64:128, qsl], start=True, stop=True)
                eab = eabp.tile([128, 2 * QP], BF16, tag="eab")
                nc.scalar.activation(eab[:], pab[:], AF.Exp, scale=SCALE)
                eabs[g] = eab
                acc = accs[p]
                if c == 0:
                    # ghostmax: +1/128 per partition carries the +1
                    nc.vector.tensor_scalar(acc[:], eab[:], 1.0 / 128.0,
                                            None, ALU.add)
                else:
                    nc.vector.tensor_tensor(acc[:], acc[:], eab[:], ALU.add)

            def emit_av(g):
                p, c = g // NCH, g % NCH
                h, _ = passes[p]
                csl = slice(c * 128, (c + 1) * 128)
                o01 = o01s[p]
                eab = eabs.pop(g)
                nc.tensor.matmul(o01[:, 0:QP], vts[h][:, csl], eab[:, 0:QP],
                                 start=(c == 0), stop=(c == NCH - 1))
                nc.tensor.matmul(o01[:, QP:2 * QP], vts[h][:, csl],
                                 eab[:, QP:2 * QP],
                                 start=(c == 0), stop=(c == NCH - 1))

            for g in range(G + 2):
                if g < G:
                    emit_scores_exp_acc(g)
                if g >= 2:
                    emit_av(g - 2)
                    gp = g - 2
                    if gp % NCH == NCH - 1:
                        p = gp // NCH
                        h, qp = passes[p]
                        pending.extend(
                            make_epilogue(h, qp, accs.pop(p), o01s.pop(p)))
                        if qp == NPASS - 1:
                            pending.extend(make_gn(h))
                    if pending:
                        pending.pop(0)()
            for f in pending:
                f()

    nc.finalize()
    return nc


def _get_program():
    global _PROGRAM
    if _PROGRAM is None:
        _PROGRAM = _build_program()
    return _PROGRAM


def _prepare_in_maps(q, k, v, lambda_q1, lambda_k1, lambda_q2, lambda_k2,
                     gn_weight, gn_bias):
    q = np.asarray(q)
    k = np.asarray(k)
    v = np.asarray(v)

    lam = np.concatenate([np.asarray(lambda_q1), np.asarray(lambda_k1),
                          np.asarray(lambda_q2), np.asarray(lambda_k2)]
                         ).astype(np.float32).reshape(1, 4 * D)
    w_hq = np.asarray(gn_weight, dtype=np.float32).reshape(HQ, 128)
    b_hq = np.asarray(gn_bias, dtype=np.float32).reshape(HQ, 128) \
        * (1.0 - LAMBDA_INIT)

    in_maps = []
    for core in range(NCORE):
        heads = [core * VH + i for i in range(VH)]
        qt = np.empty((VH, 128, S), dtype=ml_dtypes.bfloat16)
        kt = np.empty((VH, 128, S), dtype=ml_dtypes.bfloat16)
        vt = np.empty((VH, 128, S), dtype=ml_dtypes.bfloat16)
        wq = np.empty((1, VH * 128), dtype=np.float32)
        bq = np.empty((1, VH * 128), dtype=np.float32)
        for i, hh in enumerate(heads):
            qt[i, 0:64] = q[0, 2 * hh].T.astype(ml_dtypes.bfloat16)
            qt[i, 64:128] = q[0, 2 * hh + 1].T.astype(ml_dtypes.bfloat16)
            kt[i, 0:64] = k[0, 2 * hh].T.astype(ml_dtypes.bfloat16)
            kt[i, 64:128] = k[0, 2 * hh + 1].T.astype(ml_dtypes.bfloat16)
            vt[i] = (v[0, hh].reshape(NCH, 128, DV).transpose(1, 0, 2)
                     .reshape(128, S).astype(ml_dtypes.bfloat16))
            wq[0, i * 128:(i + 1) * 128] = w_hq[hh]
            bq[0, i * 128:(i + 1) * 128] = b_hq[hh]
        in_maps.append({"qt": qt, "kt": kt, "vt": vt, "lam": lam,
                        "wq": wq, "bq": bq})
    return in_maps


def _assemble(results):
    # device out[h] = [dv, s]; head output is [s, dv]
    out_heads = np.empty((HQ, S, DV), dtype=np.float32)
    for core in range(NCORE):
        o = results[core]["out"]                      # [VH, 128, 2048] bf16
        for i in range(VH):
            out_heads[core * VH + i] = np.asarray(o[i]).astype(np.float32).T
    x = out_heads.reshape(HQ * DV, S)                 # torch-style flatten
    return np.ascontiguousarray(x.T)[None]            # [1, S, C]


def kernel(**inputs):
    nc = _get_program()
    in_maps = _prepare_in_maps(**inputs)
    res = run_bass_kernel_spmd(nc, in_maps, list(range(NCORE)))
    return _assemble(res.results)


# revision 28
# speedup vs baseline: 1.3102x; 1.0106x over previous
"""Differential-attention + GroupNorm Trainium2 kernel, 8-core head-parallel.

Problem (hardcoded):
  q, k: [1, 32, 2048, 64] f32 ; v: [1, 16, 2048, 128] f32
  lambda_q1/k1/q2/k2: [64] f32 ; gn_weight/gn_bias: [2048] f32
  out:  [1, 2048, 2048] f32

Sharding: 2 v-heads (= 4 q/k heads) per core across 8 cores.

Per core the work is a flat stream of 128 key-chunk steps (2 heads x 4
query-passes x 16 chunks). Scores for both difference maps land in a
keys-on-partitions PSUM tile ([128 k, 512 q0 | 512 q1]); exp on the scalar
engine is the binding resource (128 x ~1.04us activations), so the PE queue
is software-pipelined with the AV matmuls lagging the scores by two steps —
the exp stream never waits on the chunk chain. Ghostmax denominators are
accumulated elementwise on DVE (fp16, +1/128 seed carries the ghost logit)
and column-reduced with one gpsimd partition_all_reduce per pass. The
combine (o0/d0 - lambda*o1/d1) happens post-AV in the [dv, q] layout with
per-column reciprocal rows (no PE transposes, no PSUM round trips);
GroupNorm statistics ride along as fused accum outputs, 1/sigma comes from
a magic-constant Newton rsqrt on DVE (no activation-table switch), and the
normalization is applied per head in [dv, q] with broadcast-AP coefficient
rows. The host transposes each head's [dv, q] tile when assembling.

Device inputs per core:
  qt  [2, 128, 2048] bf16 : rows 0:64 = q[2h]^T, rows 64:128 = q[2h+1]^T
  kt  [2, 128, 2048] bf16 : same for k
  vt  [2, 128, 2048] bf16 : vt[p, 128c+j] = v[h, 128c+p, j]
  lam [1, 256]       f32  : lambda_q1 | lambda_k1 | lambda_q2 | lambda_k2
  wq  [1, 256]       f32  : gn_weight per (head, s//16)
  bq  [1, 256]       f32  : gn_bias * (1-LAMBDA_INIT), same layout
Output:
  out [2, 128, 2048] bf16 : per head, out[h][d, s] (channels x positions)
"""
import math
import numpy as np
import ml_dtypes

import concourse.bass as bass
import concourse.mybir as mybir
import concourse.tile as tile
import concourse.bass_isa as bass_isa
from concourse import bacc
from concourse.bass_utils import run_bass_kernel_spmd

F32 = mybir.dt.float32
FP16 = mybir.dt.float16
BF16 = mybir.dt.bfloat16
I32 = mybir.dt.int32
AF = mybir.ActivationFunctionType
ALU = mybir.AluOpType
RED = bass_isa.ReduceOp

S = 2048          # sequence length
D = 64            # head dim of q/k
DV = 128          # head dim of v
HQ = 16           # number of v-heads
NCORE = 8
VH = HQ // NCORE  # v-heads per core = 2
QP = 512          # queries per pass
NPASS = S // QP   # 4
NCH = S // 128    # 16 key chunks
NP = VH * NPASS   # 8 passes
G = NP * NCH      # 128 global chunk steps
LAMBDA_INIT = 0.8
EPS = 1e-5
SCALE = 1.0 / math.sqrt(D)
MAGIC = 0x5F3759DF

_PROGRAM = None


def _build_program():
    nc = bacc.Bacc("TRN2", target_bir_lowering=False, debug=False,
                   num_devices=NCORE)
    qt_d = nc.dram_tensor("qt", [VH, 128, S], BF16, kind="ExternalInput").ap()
    kt_d = nc.dram_tensor("kt", [VH, 128, S], BF16, kind="ExternalInput").ap()
    vt_d = nc.dram_tensor("vt", [VH, 128, S], BF16, kind="ExternalInput").ap()
    lam_d = nc.dram_tensor("lam", [1, 4 * D], F32, kind="ExternalInput").ap()
    wq_d = nc.dram_tensor("wq", [1, VH * 128], F32, kind="ExternalInput").ap()
    bq_d = nc.dram_tensor("bq", [1, VH * 128], F32, kind="ExternalInput").ap()
    out_d = nc.dram_tensor("out", [VH, 128, S], BF16, kind="ExternalOutput").ap()

    inv_n = 1.0 / float(S * DV)

    with tile.TileContext(nc) as tc:
        with tc.tile_pool(name="const", bufs=1) as const, \
             tc.tile_pool(name="inp", bufs=1) as inp, \
             tc.tile_pool(name="eabp", bufs=8) as eabp, \
             tc.tile_pool(name="accp", bufs=2) as accp, \
             tc.tile_pool(name="dp", bufs=2) as dp, \
             tc.tile_pool(name="work", bufs=2) as work, \
             tc.tile_pool(name="sm", bufs=1) as sm, \
             tc.tile_pool(name="ps", bufs=2, space="PSUM") as ps, \
             tc.tile_pool(name="pso", bufs=2, space="PSUM") as pso:

            # ---- PE p-state warm-up (independent of inputs) ----
            wsc = const.tile([128, 512], BF16, tag="wsc")
            nc.gpsimd.memset(wsc[:], 0.5)
            for _w in range(6):
                wps = ps.tile([128, 1024], F32, tag="pab")
                nc.tensor.matmul(wps[:, 0:512], wsc[:, 0:128], wsc[:],
                                 start=True, stop=True)

            # ---- inputs; head 0 split fine-grained so chunk 0 starts early,
            # qt on the Activation DGE queue to parallelize the ramp ----
            qts, kts, vts = [], [], []
            for h in range(VH):
                kt = inp.tile([128, S], BF16, tag=f"kt{h}")
                qt = inp.tile([128, S], BF16, tag=f"qt{h}")
                vt = inp.tile([128, S], BF16, tag=f"vt{h}")
                qts.append(qt)
                kts.append(kt)
                vts.append(vt)
            lam = inp.tile([1, 4 * D], F32, tag="lam")
            wqr = inp.tile([1, VH * 128], F32, tag="wqr")
            bqr = inp.tile([1, VH * 128], F32, tag="bqr")
            # SP queue: head-0 k/v first (split so chunk 0 starts early),
            # then the small param rows, then head 1.
            nc.sync.dma_start(kts[0][:, 0:512], kt_d[0][:, 0:512])
            nc.sync.dma_start(vts[0][:, 0:512], vt_d[0][:, 0:512])
            nc.sync.dma_start(wqr[:], wq_d[:])
            nc.sync.dma_start(bqr[:], bq_d[:])
            nc.sync.dma_start(kts[0][:, 512:S], kt_d[0][:, 512:S])
            nc.sync.dma_start(vts[0][:, 512:S], vt_d[0][:, 512:S])
            nc.sync.dma_start(kts[1][:], kt_d[1])
            nc.sync.dma_start(vts[1][:], vt_d[1])
            # Activation queue (parallel): q and lambda
            nc.scalar.dma_start(qts[0][:, 0:512], qt_d[0][:, 0:512])
            nc.scalar.dma_start(lam[:], lam_d[:])
            nc.scalar.dma_start(qts[0][:, 512:S], qt_d[0][:, 512:S])
            nc.scalar.dma_start(qts[1][:], qt_d[1])

            invlamv = const.tile([128, 1], F32, tag="invlamv")
            wqb, bqb = [], []
            for h in range(VH):
                wb = const.tile([128, 128], F32, tag=f"wqb{h}")
                bb = const.tile([128, 128], F32, tag=f"bqb{h}")
                wqb.append(wb)
                bqb.append(bb)

            def make_prep():
                def prep():
                    # lambda_full = exp(lq1.lk1) - exp(lq2.lk2) + 0.8
                    scr = sm.tile([1, D], F32, tag="lscr")
                    s12 = sm.tile([1, 2], F32, tag="ls12")
                    nc.vector.tensor_tensor(scr[:], lam[:, 0:D],
                                            lam[:, D:2 * D], ALU.mult)
                    nc.vector.tensor_reduce(s12[:, 0:1], scr[:],
                                            mybir.AxisListType.X, ALU.add)
                    nc.vector.tensor_tensor(scr[:], lam[:, 2 * D:3 * D],
                                            lam[:, 3 * D:4 * D], ALU.mult)
                    nc.vector.tensor_reduce(s12[:, 1:2], scr[:],
                                            mybir.AxisListType.X, ALU.add)
                    e12 = sm.tile([1, 2], F32, tag="le12")
                    nc.scalar.activation(e12[:], s12[:], AF.Exp)
                    lamf = sm.tile([1, 1], F32, tag="lamf")
                    nc.vector.tensor_tensor(lamf[:], e12[:, 0:1], e12[:, 1:2],
                                            ALU.subtract)
                    nc.vector.tensor_scalar(lamf[:], lamf[:], LAMBDA_INIT,
                                            None, ALU.add)
                    rlamf = sm.tile([1, 1], F32, tag="rlamf")
                    nc.vector.reciprocal(rlamf[:], lamf[:])
                    nc.gpsimd.partition_broadcast(invlamv[:], rlamf[:])
                    for h in range(VH):
                        nc.gpsimd.partition_broadcast(
                            wqb[h][:], wqr[:, h * 128:(h + 1) * 128])
                        nc.gpsimd.partition_broadcast(
                            bqb[h][:], bqr[:, h * 128:(h + 1) * 128])
                return prep

            octs, sums, sqs, zs = [], [], [], []
            for h in range(VH):
                oct_t = inp.tile([128, S], BF16, tag=f"oct{h}")
                sums_t = inp.tile([128, NPASS], F32, tag=f"sums{h}")
                sqs_t = inp.tile([128, NPASS], F32, tag=f"sqs{h}")
                z_t = inp.tile([128, S], BF16, tag=f"z{h}")
                octs.append(oct_t)
                sums.append(sums_t)
                sqs.append(sqs_t)
                zs.append(z_t)

            def make_epilogue(h, qp, acc, o01):
                qsl = slice(qp * QP, (qp + 1) * QP)

                dts = []

                def epi_a():
                    # column sums per map half on Pool (d0 first, so its
                    # reciprocal can overlap d1's all_reduce); fold 1/lambda
                    # into acc's d1 half so r1 = lambda/d1
                    dt = dp.tile([128, 2 * QP], F32, tag="dt")
                    rt = dp.tile([128, 2 * QP], F32, tag="rt")
                    dts.append((dt, rt))
                    nc.gpsimd.partition_all_reduce(dt[:, 0:QP], acc[:, 0:QP],
                                                   128, RED.add)
                    nc.vector.tensor_scalar(acc[:, QP:2 * QP],
                                            acc[:, QP:2 * QP],
                                            invlamv[:], None, ALU.mult)
                    nc.gpsimd.partition_all_reduce(dt[:, QP:2 * QP],
                                                   acc[:, QP:2 * QP],
                                                   128, RED.add)

                def epi_b():
                    dt, rt = dts[0]
                    t0 = work.tile([128, QP], F32, tag="t0")
                    nc.vector.reciprocal(rt[:, 0:QP], dt[:, 0:QP])
                    nc.vector.tensor_tensor(t0[:], o01[:, 0:QP],
                                            rt[:, 0:QP], ALU.mult)
                    dts.append(t0)

                def epi_c():
                    dt, rt = dts[0]
                    t0 = dts[1]
                    t1 = work.tile([128, QP], F32, tag="t1")
                    nc.vector.reciprocal(rt[:, QP:2 * QP], dt[:, QP:2 * QP])
                    nc.vector.tensor_tensor(t1[:], o01[:, QP:2 * QP],
                                            rt[:, QP:2 * QP], ALU.mult)
                    nc.vector.scalar_tensor_tensor(
                        octs[h][:, qsl], t0[:], 1.0, t1[:],
                        ALU.mult, ALU.subtract,
                        accum_out=sums[h][:, qp:qp + 1])

                def epi_d():
                    scr2 = work.tile([128, QP], BF16, tag="scr2")
                    nc.vector.scalar_tensor_tensor(
                        scr2[:], octs[h][:, qsl], 1.0, octs[h][:, qsl],
                        ALU.mult, ALU.mult,
                        accum_out=sqs[h][:, qp:qp + 1])

                def epi_z():
                    # z = oct * w[q>>4]; lets the final apply be one fused op
                    nc.vector.tensor_tensor(
                        zs[h][:, qsl].rearrange("p (c s) -> p c s", c=32),
                        octs[h][:, qsl].rearrange("p (c s) -> p c s", c=32),
                        wqb[h][:, qp * 32:(qp + 1) * 32]
                            .rearrange("p (c one) -> p c one", one=1)
                            .broadcast_to([128, 32, 16]),
                        ALU.mult)

                return [epi_a, epi_b, epi_c, epi_z, epi_d]

            def make_gn(h):
                def gn_stats():
                    st = sm.tile([128, 4], F32, tag="st")
                    nc.vector.tensor_reduce(st[:, 0:1], sums[h][:],
                                            mybir.AxisListType.X, ALU.add)
                    nc.vector.tensor_reduce(st[:, 1:2], sqs[h][:],
                                            mybir.AxisListType.X, ALU.add)
                    nc.gpsimd.partition_all_reduce(st[:, 0:2], st[:, 0:2],
                                                   128, RED.add)
                    mu = sm.tile([128, 4], F32, tag="mu")
                    nc.vector.tensor_scalar(mu[:, 0:1], st[:, 0:1], inv_n,
                                            None, ALU.mult)            # mu
                    nc.vector.tensor_scalar(mu[:, 1:2], st[:, 1:2], inv_n,
                                            EPS, ALU.mult, ALU.add)  # E2+eps
                    nc.vector.tensor_scalar(mu[:, 3:4], mu[:, 0:1], -1.0,
                                            None, ALU.mult)            # -mu
                    nc.vector.scalar_tensor_tensor(
                        mu[:, 2:3], mu[:, 0:1], mu[:, 3:4], mu[:, 1:2],
                        ALU.mult, ALU.add)       # var+eps = E2+eps - mu^2
                    # 1/sigma: magic-constant rsqrt + 2 Newton steps on DVE
                    sh = sm.tile([128, 1], I32, tag="sh")
                    nc.vector.tensor_scalar(sh[:], mu[:, 2:3].bitcast(I32),
                                            1, None, ALU.logical_shift_right)
                    nc.vector.tensor_scalar(sh[:], sh[:], -1, MAGIC,
                                            ALU.mult, ALU.add)
                    y = sm.tile([128, 1], F32, tag="y")
                    nc.vector.tensor_copy(y[:].bitcast(I32), sh[:])
                    t = sm.tile([128, 1], F32, tag="t")
                    hna = sm.tile([128, 1], F32, tag="hna")
                    nc.vector.tensor_scalar(hna[:], mu[:, 2:3], -0.5, None,
                                            ALU.mult)
                    for _ in range(2):
                        nc.vector.tensor_tensor(t[:], y[:], y[:], ALU.mult)
                        nc.vector.tensor_scalar(t[:], t[:], hna[:], 1.5,
                                                ALU.mult, ALU.add)
                        nc.vector.tensor_tensor(y[:], y[:], t[:], ALU.mult)
                    i02 = sm.tile([128, 1], F32, tag="i02")
                    nc.vector.tensor_scalar(i02[:], y[:],
                                            1.0 - LAMBDA_INIT, None, ALU.mult)
                    # C = b*0.2 - mu*i02*w  (z*i02 + C is the full apply)
                    nmi = sm.tile([128, 1], F32, tag="nmi")
                    nc.vector.tensor_tensor(nmi[:], mu[:, 3:4], i02[:],
                                            ALU.mult)
                    cc = sm.tile([128, 128], F32, tag=f"cc{h}")
                    nc.vector.scalar_tensor_tensor(
                        cc[:], wqb[h][:], nmi[:], bqb[h][:],
                        ALU.mult, ALU.add)
                    return i02, cc

                coeffs = []
                pieces = [lambda: coeffs.append(gn_stats())]

                # out = z * (1/sigma * 0.2) + C[q>>4]: one fused op per
                # quarter, DMAs overlap the remaining applies
                outf = inp.tile([128, S], BF16, tag=f"outf{h}")

                def make_quarter(qu):
                    def quarter():
                        i02, cc = coeffs[0]
                        qs = slice(qu * (S // 4), (qu + 1) * (S // 4))
                        qa = slice(qu * 32, (qu + 1) * 32)
                        nc.vector.scalar_tensor_tensor(
                            outf[:, qs].rearrange("p (c s) -> p c s", c=32),
                            zs[h][:, qs].rearrange("p (c s) -> p c s", c=32),
                            i02[:],
                            cc[:, qa].rearrange("p (c one) -> p c one", one=1)
                                .broadcast_to([128, 32, 16]),
                            ALU.mult, ALU.add)
                        nc.sync.dma_start(out_d[h, :, qs], outf[:, qs])
                    return quarter

                for qu in range(4):
                    pieces.append(make_quarter(qu))
                return pieces

            # ---- main pipeline: flat over 128 global chunk steps ----
            passes = [(h, qp) for h in range(VH) for qp in range(NPASS)]
            accs = {}
            o01s = {}
            eabs = {}
            pending = [make_prep()]  # queue of small emitters, 1 per step

            def emit_scores_exp_acc(g):
                p, c = g // NCH, g % NCH
                h, qp = passes[p]
                qsl = slice(qp * QP, (qp + 1) * QP)
                csl = slice(c * 128, (c + 1) * 128)
                if c == 0:
                    acc = accp.tile([128, 2 * QP], FP16, tag="acc")
                    o01 = pso.tile([128, 2 * QP], F32, tag="o01")
                    accs[p] = acc
                    o01s[p] = o01
                pab = ps.tile([128, 2 * QP], F32, tag="pab")
                nc.tensor.matmul(pab[:, 0:QP], kts[h][0:64, csl],
                                 qts[h][0:64, qsl], start=True, stop=True)
                nc.tensor.matmul(pab[:, QP:2 * QP], kts[h][64:128, csl],
                                 qts[h][64:128, qsl], start=True, stop=True)
                eab = eabp.tile([128, 2 * QP], BF16, tag="eab")
                nc.scalar.activation(eab[:], pab[:], AF.Exp, scale=SCALE)
                eabs[g] = eab
                acc = accs[p]
                if c == 0:
                    # ghostmax: +1/128 per partition carries the +1
                    nc.vector.tensor_scalar(acc[:], eab[:], 1.0 / 128.0,
                                            None, ALU.add)
                else:
                    nc.vector.tensor_tensor(acc[:], acc[:], eab[:], ALU.add)

            def emit_av(g):
                p, c = g // NCH, g % NCH
                h, _ = passes[p]
                csl = slice(c * 128, (c + 1) * 128)
                o01 = o01s[p]
                eab = eabs.pop(g)
                nc.tensor.matmul(o01[:, 0:QP], vts[h][:, csl], eab[:, 0:QP],
                                 start=(c == 0), stop=(c == NCH - 1))
                nc.tensor.matmul(o01[:, QP:2 * QP], vts[h][:, csl],
                                 eab[:, QP:2 * QP],
                                 start=(c == 0), stop=(c == NCH - 1))

            for g in range(G + 2):
                if g < G:
                    emit_scores_exp_acc(g)
                if g >= 2:
                    emit_av(g - 2)
                    gp = g - 2
                    if gp % NCH == NCH - 1:
                        p = gp // NCH
                        h, qp = passes[p]
                        pending.extend(
                            make_epilogue(h, qp, accs.pop(p), o01s.pop(p)))
                        if qp == NPASS - 1:
                            pending.extend(make_gn(h))
                    if pending:
                        pending.pop(0)()
            for f in pending:
                f()

    nc.finalize()
    return nc


def _get_program():
    global _PROGRAM
    if _PROGRAM is None:
        _PROGRAM = _build_program()
    return _PROGRAM


def _prepare_in_maps(q, k, v, lambda_q1, lambda_k1, lambda_q2, lambda_k2,
                     gn_weight, gn_bias):
    q = np.asarray(q)
    k = np.asarray(k)
    v = np.asarray(v)

    lam = np.concatenate([np.asarray(lambda_q1), np.asarray(lambda_k1),
                          np.asarray(lambda_q2), np.asarray(lambda_k2)]
                         ).astype(np.float32).reshape(1, 4 * D)
    w_hq = np.asarray(gn_weight, dtype=np.float32).reshape(HQ, 128)
    b_hq = np.asarray(gn_bias, dtype=np.float32).reshape(HQ, 128) \
        * (1.0 - LAMBDA_INIT)

    in_maps = []
    for core in range(NCORE):
        heads = [core * VH + i for i in range(VH)]
        qt = np.empty((VH, 128, S), dtype=ml_dtypes.bfloat16)
        kt = np.empty((VH, 128, S), dtype=ml_dtypes.bfloat16)
        vt = np.empty((VH, 128, S), dtype=ml_dtypes.bfloat16)
        wq = np.empty((1, VH * 128), dtype=np.float32)
        bq = np.empty((1, VH * 128), dtype=np.float32)
        for i, hh in enumerate(heads):
            qt[i, 0:64] = q[0, 2 * hh].T.astype(ml_dtypes.bfloat16)
            qt[i, 64:128] = q[0, 2 * hh + 1].T.astype(ml_dtypes.bfloat16)
            kt[i, 0:64] = k[0, 2 * hh].T.astype(ml_dtypes.bfloat16)
            kt[i, 64:128] = k[0, 2 * hh + 1].T.astype(ml_dtypes.bfloat16)
            vt[i] = (v[0, hh].reshape(NCH, 128, DV).transpose(1, 0, 2)
                     .reshape(128, S).astype(ml_dtypes.bfloat16))
            wq[0, i * 128:(i + 1) * 128] = w_hq[hh]
            bq[0, i * 128:(i + 1) * 128] = b_hq[hh]
        in_maps.append({"qt": qt, "kt": kt, "vt": vt, "lam": lam,
                        "wq": wq, "bq": bq})
    return in_maps


def _assemble(results):
    # device out[h] = [dv, s]; head output is [s, dv]
    out_heads = np.empty((HQ, S, DV), dtype=np.float32)
    for core in range(NCORE):
        o = results[core]["out"]                      # [VH, 128, 2048] bf16
        for i in range(VH):
            out_heads[core * VH + i] = np.asarray(o[i]).astype(np.float32).T
    x = out_heads.reshape(HQ * DV, S)                 # torch-style flatten
    return np.ascontiguousarray(x.T)[None]            # [1, S, C]


def kernel(**inputs):
    nc = _get_program()
    in_maps = _prepare_in_maps(**inputs)
    res = run_bass_kernel_spmd(nc, in_maps, list(range(NCORE)))
    return _assemble(res.results)


# revision 29
# speedup vs baseline: 1.3124x; 1.0017x over previous
"""Differential-attention + GroupNorm Trainium2 kernel, 8-core head-parallel.

Problem (hardcoded):
  q, k: [1, 32, 2048, 64] f32 ; v: [1, 16, 2048, 128] f32
  lambda_q1/k1/q2/k2: [64] f32 ; gn_weight/gn_bias: [2048] f32
  out:  [1, 2048, 2048] f32

Sharding: 2 v-heads (= 4 q/k heads) per core across 8 cores.

Per core the work is a flat stream of 128 key-chunk steps (2 heads x 4
query-passes x 16 chunks). Scores for both difference maps land in a
keys-on-partitions PSUM tile ([128 k, 512 q0 | 512 q1]); exp on the scalar
engine is the binding resource (128 x ~1.04us activations), so the PE queue
is software-pipelined with the AV matmuls lagging the scores by two steps —
the exp stream never waits on the chunk chain. Ghostmax denominators are
accumulated elementwise on DVE (fp16, +1/128 seed carries the ghost logit)
and column-reduced with one gpsimd partition_all_reduce per pass. The
combine (o0/d0 - lambda*o1/d1) happens post-AV in the [dv, q] layout with
per-column reciprocal rows (no PE transposes, no PSUM round trips);
GroupNorm statistics ride along as fused accum outputs, 1/sigma comes from
a magic-constant Newton rsqrt on DVE (no activation-table switch), and the
normalization is applied per head in [dv, q] with broadcast-AP coefficient
rows. The host transposes each head's [dv, q] tile when assembling.

Device inputs per core:
  qt  [2, 128, 2048] bf16 : rows 0:64 = q[2h]^T, rows 64:128 = q[2h+1]^T
  kt  [2, 128, 2048] bf16 : same for k
  vt  [2, 128, 2048] bf16 : vt[p, 128c+j] = v[h, 128c+p, j]
  lam [1, 256]       f32  : lambda_q1 | lambda_k1 | lambda_q2 | lambda_k2
  wq  [1, 256]       f32  : gn_weight per (head, s//16)
  bq  [1, 256]       f32  : gn_bias * (1-LAMBDA_INIT), same layout
Output:
  out [2, 128, 2048] bf16 : per head, out[h][d, s] (channels x positions)
"""
import math
import numpy as np
import ml_dtypes

import concourse.bass as bass
import concourse.mybir as mybir
import concourse.tile as tile
import concourse.bass_isa as bass_isa
from concourse import bacc
from concourse.bass_utils import run_bass_kernel_spmd

F32 = mybir.dt.float32
FP16 = mybir.dt.float16
BF16 = mybir.dt.bfloat16
I32 = mybir.dt.int32
AF = mybir.ActivationFunctionType
ALU = mybir.AluOpType
RED = bass_isa.ReduceOp

S = 2048          # sequence length
D = 64            # head dim of q/k
DV = 128          # head dim of v
HQ = 16           # number of v-heads
NCORE = 8
VH = HQ // NCORE  # v-heads per core = 2
QP = 512          # queries per pass
NPASS = S // QP   # 4
NCH = S // 128    # 16 key chunks
NP = VH * NPASS   # 8 passes
G = NP * NCH      # 128 global chunk steps
LAMBDA_INIT = 0.8
EPS = 1e-5
SCALE = 1.0 / math.sqrt(D)
MAGIC = 0x5F3759DF

_PROGRAM = None


def _build_program():
    nc = bacc.Bacc("TRN2", target_bir_lowering=False, debug=False,
                   num_devices=NCORE)
    qt_d = nc.dram_tensor("qt", [VH, 128, S], BF16, kind="ExternalInput").ap()
    kt_d = nc.dram_tensor("kt", [VH, 128, S], BF16, kind="ExternalInput").ap()
    vt_d = nc.dram_tensor("vt", [VH, 128, S], BF16, kind="ExternalInput").ap()
    lam_d = nc.dram_tensor("lam", [1, 4 * D], F32, kind="ExternalInput").ap()
    wq_d = nc.dram_tensor("wq", [1, VH * 128], F32, kind="ExternalInput").ap()
    bq_d = nc.dram_tensor("bq", [1, VH * 128], F32, kind="ExternalInput").ap()
    out_d = nc.dram_tensor("out", [VH, 128, S], BF16, kind="ExternalOutput").ap()

    inv_n = 1.0 / float(S * DV)

    with tile.TileContext(nc) as tc:
        with tc.tile_pool(name="const", bufs=1) as const, \
             tc.tile_pool(name="inp", bufs=1) as inp, \
             tc.tile_pool(name="eabp", bufs=8) as eabp, \
             tc.tile_pool(name="accp", bufs=2) as accp, \
             tc.tile_pool(name="dp", bufs=2) as dp, \
             tc.tile_pool(name="work", bufs=2) as work, \
             tc.tile_pool(name="sm", bufs=1) as sm, \
             tc.tile_pool(name="ps", bufs=2, space="PSUM") as ps, \
             tc.tile_pool(name="pso", bufs=2, space="PSUM") as pso:

            # ---- PE p-state warm-up (independent of inputs) ----
            wsc = const.tile([128, 512], BF16, tag="wsc")
            nc.gpsimd.memset(wsc[:], 0.5)
            for _w in range(6):
                wps = ps.tile([128, 1024], F32, tag="pab")
                nc.tensor.matmul(wps[:, 0:512], wsc[:, 0:128], wsc[:],
                                 start=True, stop=True)

            # ---- inputs; head 0 split fine-grained so chunk 0 starts early,
            # qt on the Activation DGE queue to parallelize the ramp ----
            qts, kts, vts = [], [], []
            for h in range(VH):
                kt = inp.tile([128, S], BF16, tag=f"kt{h}")
                qt = inp.tile([128, S], BF16, tag=f"qt{h}")
                vt = inp.tile([128, S], BF16, tag=f"vt{h}")
                qts.append(qt)
                kts.append(kt)
                vts.append(vt)
            lam = inp.tile([1, 4 * D], F32, tag="lam")
            wqr = inp.tile([1, VH * 128], F32, tag="wqr")
            bqr = inp.tile([1, VH * 128], F32, tag="bqr")
            # SP queue: head-0 k/v first (split so chunk 0 starts early),
            # then the small param rows, then head 1.
            nc.sync.dma_start(kts[0][:, 0:512], kt_d[0][:, 0:512])
            nc.sync.dma_start(vts[0][:, 0:512], vt_d[0][:, 0:512])
            nc.sync.dma_start(wqr[:], wq_d[:])
            nc.sync.dma_start(bqr[:], bq_d[:])
            nc.sync.dma_start(kts[0][:, 512:S], kt_d[0][:, 512:S])
            nc.sync.dma_start(vts[0][:, 512:S], vt_d[0][:, 512:S])
            nc.sync.dma_start(kts[1][:], kt_d[1])
            nc.sync.dma_start(vts[1][:], vt_d[1])
            # Activation queue (parallel): q and lambda
            nc.scalar.dma_start(qts[0][:, 0:512], qt_d[0][:, 0:512])
            nc.scalar.dma_start(lam[:], lam_d[:])
            nc.scalar.dma_start(qts[0][:, 512:S], qt_d[0][:, 512:S])
            nc.scalar.dma_start(qts[1][:], qt_d[1])

            invlamv = const.tile([128, 1], F32, tag="invlamv")
            wqb, bqb = [], []
            for h in range(VH):
                wb = const.tile([128, 128], F32, tag=f"wqb{h}")
                bb = const.tile([128, 128], F32, tag=f"bqb{h}")
                wqb.append(wb)
                bqb.append(bb)

            def make_prep():
                def prep():
                    # lambda_full = exp(lq1.lk1) - exp(lq2.lk2) + 0.8
                    scr = sm.tile([1, D], F32, tag="lscr")
                    s12 = sm.tile([1, 2], F32, tag="ls12")
                    nc.vector.tensor_tensor(scr[:], lam[:, 0:D],
                                            lam[:, D:2 * D], ALU.mult)
                    nc.vector.tensor_reduce(s12[:, 0:1], scr[:],
                                            mybir.AxisListType.X, ALU.add)
                    nc.vector.tensor_tensor(scr[:], lam[:, 2 * D:3 * D],
                                            lam[:, 3 * D:4 * D], ALU.mult)
                    nc.vector.tensor_reduce(s12[:, 1:2], scr[:],
                                            mybir.AxisListType.X, ALU.add)
                    e12 = sm.tile([1, 2], F32, tag="le12")
                    nc.scalar.activation(e12[:], s12[:], AF.Exp)
                    lamf = sm.tile([1, 1], F32, tag="lamf")
                    nc.vector.tensor_tensor(lamf[:], e12[:, 0:1], e12[:, 1:2],
                                            ALU.subtract)
                    nc.vector.tensor_scalar(lamf[:], lamf[:], LAMBDA_INIT,
                                            None, ALU.add)
                    rlamf = sm.tile([1, 1], F32, tag="rlamf")
                    nc.vector.reciprocal(rlamf[:], lamf[:])
                    nc.gpsimd.partition_broadcast(invlamv[:], rlamf[:])
                    for h in range(VH):
                        nc.gpsimd.partition_broadcast(
                            wqb[h][:], wqr[:, h * 128:(h + 1) * 128])
                        nc.gpsimd.partition_broadcast(
                            bqb[h][:], bqr[:, h * 128:(h + 1) * 128])
                return prep

            octs, sums, sqs, zs = [], [], [], []
            for h in range(VH):
                oct_t = inp.tile([128, S], BF16, tag=f"oct{h}")
                sums_t = inp.tile([128, NPASS], F32, tag=f"sums{h}")
                sqs_t = inp.tile([128, NPASS], F32, tag=f"sqs{h}")
                z_t = inp.tile([128, S], BF16, tag=f"z{h}")
                octs.append(oct_t)
                sums.append(sums_t)
                sqs.append(sqs_t)
                zs.append(z_t)

            def make_epilogue(h, qp, acc, o01):
                qsl = slice(qp * QP, (qp + 1) * QP)

                dts = []

                def epi_a():
                    # column sums per map half on Pool (d0 first, so its
                    # reciprocal can overlap d1's all_reduce); fold 1/lambda
                    # into acc's d1 half so r1 = lambda/d1
                    dt = dp.tile([128, 2 * QP], F32, tag="dt")
                    rt = dp.tile([128, 2 * QP], F32, tag="rt")
                    dts.append((dt, rt))
                    nc.gpsimd.partition_all_reduce(dt[:, 0:QP], acc[:, 0:QP],
                                                   128, RED.add)
                    nc.vector.tensor_scalar(acc[:, QP:2 * QP],
                                            acc[:, QP:2 * QP],
                                            invlamv[:], None, ALU.mult)
                    nc.gpsimd.partition_all_reduce(dt[:, QP:2 * QP],
                                                   acc[:, QP:2 * QP],
                                                   128, RED.add)

                def epi_b():
                    dt, rt = dts[0]
                    t0 = work.tile([128, QP], F32, tag="t0")
                    nc.vector.reciprocal(rt[:, 0:QP], dt[:, 0:QP])
                    nc.vector.tensor_tensor(t0[:], o01[:, 0:QP],
                                            rt[:, 0:QP], ALU.mult)
                    dts.append(t0)

                def epi_c():
                    dt, rt = dts[0]
                    t0 = dts[1]
                    t1 = work.tile([128, QP], F32, tag="t1")
                    nc.vector.reciprocal(rt[:, QP:2 * QP], dt[:, QP:2 * QP])
                    nc.vector.tensor_tensor(t1[:], o01[:, QP:2 * QP],
                                            rt[:, QP:2 * QP], ALU.mult)
                    nc.vector.scalar_tensor_tensor(
                        octs[h][:, qsl], t0[:], 1.0, t1[:],
                        ALU.mult, ALU.subtract,
                        accum_out=sums[h][:, qp:qp + 1])

                def epi_d():
                    scr2 = work.tile([128, QP], BF16, tag="scr2")
                    nc.vector.scalar_tensor_tensor(
                        scr2[:], octs[h][:, qsl], 1.0, octs[h][:, qsl],
                        ALU.mult, ALU.mult,
                        accum_out=sqs[h][:, qp:qp + 1])

                def epi_z():
                    # z = oct * w[q>>4]; lets the final apply be one fused op
                    nc.gpsimd.tensor_tensor(
                        zs[h][:, qsl].rearrange("p (c s) -> p c s", c=32),
                        octs[h][:, qsl].rearrange("p (c s) -> p c s", c=32),
                        wqb[h][:, qp * 32:(qp + 1) * 32]
                            .rearrange("p (c one) -> p c one", one=1)
                            .broadcast_to([128, 32, 16]),
                        ALU.mult)

                return [epi_a, epi_b, epi_c, epi_z, epi_d]

            def make_gn(h):
                def gn_stats():
                    st = sm.tile([128, 4], F32, tag="st")
                    nc.vector.tensor_reduce(st[:, 0:1], sums[h][:],
                                            mybir.AxisListType.X, ALU.add)
                    nc.vector.tensor_reduce(st[:, 1:2], sqs[h][:],
                                            mybir.AxisListType.X, ALU.add)
                    nc.gpsimd.partition_all_reduce(st[:, 0:2], st[:, 0:2],
                                                   128, RED.add)
                    mu = sm.tile([128, 4], F32, tag="mu")
                    nc.vector.tensor_scalar(mu[:, 0:1], st[:, 0:1], inv_n,
                                            None, ALU.mult)            # mu
                    nc.vector.tensor_scalar(mu[:, 1:2], st[:, 1:2], inv_n,
                                            EPS, ALU.mult, ALU.add)  # E2+eps
                    nc.vector.tensor_scalar(mu[:, 3:4], mu[:, 0:1], -1.0,
                                            None, ALU.mult)            # -mu
                    nc.vector.scalar_tensor_tensor(
                        mu[:, 2:3], mu[:, 0:1], mu[:, 3:4], mu[:, 1:2],
                        ALU.mult, ALU.add)       # var+eps = E2+eps - mu^2
                    # 1/sigma: magic-constant rsqrt + 2 Newton steps on DVE
                    sh = sm.tile([128, 1], I32, tag="sh")
                    nc.vector.tensor_scalar(sh[:], mu[:, 2:3].bitcast(I32),
                                            1, None, ALU.logical_shift_right)
                    nc.vector.tensor_scalar(sh[:], sh[:], -1, MAGIC,
                                            ALU.mult, ALU.add)
                    y = sm.tile([128, 1], F32, tag="y")
                    nc.vector.tensor_copy(y[:].bitcast(I32), sh[:])
                    t = sm.tile([128, 1], F32, tag="t")
                    hna = sm.tile([128, 1], F32, tag="hna")
                    nc.vector.tensor_scalar(hna[:], mu[:, 2:3], -0.5, None,
                                            ALU.mult)
                    for _ in range(2):
                        nc.vector.tensor_tensor(t[:], y[:], y[:], ALU.mult)
                        nc.vector.tensor_scalar(t[:], t[:], hna[:], 1.5,
                                                ALU.mult, ALU.add)
                        nc.vector.tensor_tensor(y[:], y[:], t[:], ALU.mult)
                    i02 = sm.tile([128, 1], F32, tag="i02")
                    nc.vector.tensor_scalar(i02[:], y[:],
                                            1.0 - LAMBDA_INIT, None, ALU.mult)
                    # C = b*0.2 - mu*i02*w  (z*i02 + C is the full apply)
                    nmi = sm.tile([128, 1], F32, tag="nmi")
                    nc.vector.tensor_tensor(nmi[:], mu[:, 3:4], i02[:],
                                            ALU.mult)
                    cc = sm.tile([128, 128], F32, tag=f"cc{h}")
                    nc.vector.scalar_tensor_tensor(
                        cc[:], wqb[h][:], nmi[:], bqb[h][:],
                        ALU.mult, ALU.add)
                    return i02, cc

                coeffs = []
                pieces = [lambda: coeffs.append(gn_stats())]

                # out = z * (1/sigma * 0.2) + C[q>>4]: one fused op per
                # quarter, DMAs overlap the remaining applies
                outf = inp.tile([128, S], BF16, tag=f"outf{h}")

                def make_quarter(qu):
                    def quarter():
                        i02, cc = coeffs[0]
                        qs = slice(qu * (S // 4), (qu + 1) * (S // 4))
                        qa = slice(qu * 32, (qu + 1) * 32)
                        nc.vector.scalar_tensor_tensor(
                            outf[:, qs].rearrange("p (c s) -> p c s", c=32),
                            zs[h][:, qs].rearrange("p (c s) -> p c s", c=32),
                            i02[:],
                            cc[:, qa].rearrange("p (c one) -> p c one", one=1)
                                .broadcast_to([128, 32, 16]),
                            ALU.mult, ALU.add)
                        nc.sync.dma_start(out_d[h, :, qs], outf[:, qs])
                    return quarter

                for qu in range(4):
                    pieces.append(make_quarter(qu))
                return pieces

            # ---- main pipeline: flat over 128 global chunk steps ----
            passes = [(h, qp) for h in range(VH) for qp in range(NPASS)]
            accs = {}
            o01s = {}
            eabs = {}
            pending = [make_prep()]  # queue of small emitters, 1 per step

            def emit_scores_exp_acc(g):
                p, c = g // NCH, g % NCH
                h, qp = passes[p]
                qsl = slice(qp * QP, (qp + 1) * QP)
                csl = slice(c * 128, (c + 1) * 128)
                if c == 0:
                    acc = accp.tile([128, 2 * QP], FP16, tag="acc")
                    o01 = pso.tile([128, 2 * QP], F32, tag="o01")
                    accs[p] = acc
                    o01s[p] = o01
                pab = ps.tile([128, 2 * QP], F32, tag="pab")
                nc.tensor.matmul(pab[:, 0:QP], kts[h][0:64, csl],
                                 qts[h][0:64, qsl], start=True, stop=True)
                nc.tensor.matmul(pab[:, QP:2 * QP], kts[h][64:128, csl],
                                 qts[h][64:128, qsl], start=True, stop=True)
                eab = eabp.tile([128, 2 * QP], BF16, tag="eab")
                nc.scalar.activation(eab[:], pab[:], AF.Exp, scale=SCALE)
                eabs[g] = eab
                acc = accs[p]
                if c == 0:
                    # ghostmax: +1/128 per partition carries the +1
                    nc.vector.tensor_scalar(acc[:], eab[:], 1.0 / 128.0,
                                            None, ALU.add)
                else:
                    nc.vector.tensor_tensor(acc[:], acc[:], eab[:], ALU.add)

            def emit_av(g):
                p, c = g // NCH, g % NCH
                h, _ = passes[p]
                csl = slice(c * 128, (c + 1) * 128)
                o01 = o01s[p]
                eab = eabs.pop(g)
                nc.tensor.matmul(o01[:, 0:QP], vts[h][:, csl], eab[:, 0:QP],
                                 start=(c == 0), stop=(c == NCH - 1))
                nc.tensor.matmul(o01[:, QP:2 * QP], vts[h][:, csl],
                                 eab[:, QP:2 * QP],
                                 start=(c == 0), stop=(c == NCH - 1))

            for g in range(G + 2):
                if g < G:
                    emit_scores_exp_acc(g)
                if g >= 2:
                    emit_av(g - 2)
                    gp = g - 2
                    if gp % NCH == NCH - 1:
                        p = gp // NCH
                        h, qp = passes[p]
                        pending.extend(
                            make_epilogue(h, qp, accs.pop(p), o01s.pop(p)))
                        if qp == NPASS - 1:
                            pending.extend(make_gn(h))
                    if pending:
                        pending.pop(0)()
            for f in pending:
                f()

    nc.finalize()
    return nc


def _get_program():
    global _PROGRAM
    if _PROGRAM is None:
        _PROGRAM = _build_program()
    return _PROGRAM


def _prepare_in_maps(q, k, v, lambda_q1, lambda_k1, lambda_q2, lambda_k2,
                     gn_weight, gn_bias):
    q = np.asarray(q)
    k = np.asarray(k)
    v = np.asarray(v)

    lam = np.concatenate([np.asarray(lambda_q1), np.asarray(lambda_k1),
                          np.asarray(lambda_q2), np.asarray(lambda_k2)]
                         ).astype(np.float32).reshape(1, 4 * D)
    w_hq = np.asarray(gn_weight, dtype=np.float32).reshape(HQ, 128)
    b_hq = np.asarray(gn_bias, dtype=np.float32).reshape(HQ, 128) \
        * (1.0 - LAMBDA_INIT)

    in_maps = []
    for core in range(NCORE):
        heads = [core * VH + i for i in range(VH)]
        qt = np.empty((VH, 128, S), dtype=ml_dtypes.bfloat16)
        kt = np.empty((VH, 128, S), dtype=ml_dtypes.bfloat16)
        vt = np.empty((VH, 128, S), dtype=ml_dtypes.bfloat16)
        wq = np.empty((1, VH * 128), dtype=np.float32)
        bq = np.empty((1, VH * 128), dtype=np.float32)
        for i, hh in enumerate(heads):
            qt[i, 0:64] = q[0, 2 * hh].T.astype(ml_dtypes.bfloat16)
            qt[i, 64:128] = q[0, 2 * hh + 1].T.astype(ml_dtypes.bfloat16)
            kt[i, 0:64] = k[0, 2 * hh].T.astype(ml_dtypes.bfloat16)
            kt[i, 64:128] = k[0, 2 * hh + 1].T.astype(ml_dtypes.bfloat16)
            vt[i] = (v[0, hh].reshape(NCH, 128, DV).transpose(1, 0, 2)
                     .reshape(128, S).astype(ml_dtypes.bfloat16))
            wq[0, i * 128:(i + 1) * 128] = w_hq[hh]
            bq[0, i * 128:(i + 1) * 128] = b_hq[hh]
        in_maps.append({"qt": qt, "kt": kt, "vt": vt, "lam": lam,
                        "wq": wq, "bq": bq})
    return in_maps


def _assemble(results):
    # device out[h] = [dv, s]; head output is [s, dv]
    out_heads = np.empty((HQ, S, DV), dtype=np.float32)
    for core in range(NCORE):
        o = results[core]["out"]                      # [VH, 128, 2048] bf16
        for i in range(VH):
            out_heads[core * VH + i] = np.asarray(o[i]).astype(np.float32).T
    x = out_heads.reshape(HQ * DV, S)                 # torch-style flatten
    return np.ascontiguousarray(x.T)[None]            # [1, S, C]


def kernel(**inputs):
    nc = _get_program()
    in_maps = _prepare_in_maps(**inputs)
    res = run_bass_kernel_spmd(nc, in_maps, list(range(NCORE)))
    return _assemble(res.results)


# revision 32
# speedup vs baseline: 1.3128x; 1.0003x over previous
"""Differential-attention + GroupNorm Trainium2 kernel, 8-core head-parallel.

Problem (hardcoded):
  q, k: [1, 32, 2048, 64] f32 ; v: [1, 16, 2048, 128] f32
  lambda_q1/k1/q2/k2: [64] f32 ; gn_weight/gn_bias: [2048] f32
  out:  [1, 2048, 2048] f32

Sharding: 2 v-heads (= 4 q/k heads) per core across 8 cores.

Per core the work is a flat stream of 128 key-chunk steps (2 heads x 4
query-passes x 16 chunks). Scores for both difference maps land in a
keys-on-partitions PSUM tile ([128 k, 512 q0 | 512 q1]); exp on the scalar
engine is the binding resource (128 x ~1.04us activations), so the PE queue
is software-pipelined with the AV matmuls lagging the scores by two steps —
the exp stream never waits on the chunk chain. Ghostmax denominators are
accumulated elementwise on DVE (fp16, +1/128 seed carries the ghost logit)
and column-reduced with one gpsimd partition_all_reduce per pass. The
combine (o0/d0 - lambda*o1/d1) happens post-AV in the [dv, q] layout with
per-column reciprocal rows (no PE transposes, no PSUM round trips);
GroupNorm statistics ride along as fused accum outputs, 1/sigma comes from
a magic-constant Newton rsqrt on DVE (no activation-table switch), and the
normalization is applied per head in [dv, q] with broadcast-AP coefficient
rows. The host transposes each head's [dv, q] tile when assembling.

Device inputs per core:
  qt  [2, 128, 2048] bf16 : rows 0:64 = q[2h]^T, rows 64:128 = q[2h+1]^T
  kt  [2, 128, 2048] bf16 : same for k
  vt  [2, 128, 2048] bf16 : vt[p, 128c+j] = v[h, 128c+p, j]
  lam [1, 256]       f32  : lambda_q1 | lambda_k1 | lambda_q2 | lambda_k2
  wq  [1, 256]       f32  : gn_weight per (head, s//16)
  bq  [1, 256]       f32  : gn_bias * (1-LAMBDA_INIT), same layout
Output:
  out [2, 128, 2048] bf16 : per head, out[h][d, s] (channels x positions)
"""
import math
import numpy as np
import ml_dtypes

import concourse.bass as bass
import concourse.mybir as mybir
import concourse.tile as tile
import concourse.bass_isa as bass_isa
from concourse import bacc
from concourse.bass_utils import run_bass_kernel_spmd

F32 = mybir.dt.float32
FP16 = mybir.dt.float16
BF16 = mybir.dt.bfloat16
I32 = mybir.dt.int32
AF = mybir.ActivationFunctionType
ALU = mybir.AluOpType
RED = bass_isa.ReduceOp

S = 2048          # sequence length
D = 64            # head dim of q/k
DV = 128          # head dim of v
HQ = 16           # number of v-heads
NCORE = 8
VH = HQ // NCORE  # v-heads per core = 2
QP = 512          # queries per pass
NPASS = S // QP   # 4
NCH = S // 128    # 16 key chunks
NP = VH * NPASS   # 8 passes
G = NP * NCH      # 128 global chunk steps
LAMBDA_INIT = 0.8
EPS = 1e-5
SCALE = 1.0 / math.sqrt(D)
MAGIC = 0x5F3759DF

_PROGRAM = None


def _build_program():
    nc = bacc.Bacc("TRN2", target_bir_lowering=False, debug=False,
                   num_devices=NCORE)
    qt_d = nc.dram_tensor("qt", [VH, 128, S], BF16, kind="ExternalInput").ap()
    kt_d = nc.dram_tensor("kt", [VH, 128, S], BF16, kind="ExternalInput").ap()
    vt_d = nc.dram_tensor("vt", [VH, 128, S], BF16, kind="ExternalInput").ap()
    lam_d = nc.dram_tensor("lam", [1, 4 * D], F32, kind="ExternalInput").ap()
    wq_d = nc.dram_tensor("wq", [1, VH * 128], F32, kind="ExternalInput").ap()
    bq_d = nc.dram_tensor("bq", [1, VH * 128], F32, kind="ExternalInput").ap()
    out_d = nc.dram_tensor("out", [VH, 128, S], BF16, kind="ExternalOutput").ap()

    inv_n = 1.0 / float(S * DV)

    with tile.TileContext(nc) as tc:
        with tc.tile_pool(name="const", bufs=1) as const, \
             tc.tile_pool(name="inp", bufs=1) as inp, \
             tc.tile_pool(name="eabp", bufs=8) as eabp, \
             tc.tile_pool(name="accp", bufs=2) as accp, \
             tc.tile_pool(name="dp", bufs=2) as dp, \
             tc.tile_pool(name="work", bufs=2) as work, \
             tc.tile_pool(name="sm", bufs=1) as sm, \
             tc.tile_pool(name="ps", bufs=2, space="PSUM") as ps, \
             tc.tile_pool(name="pso", bufs=2, space="PSUM") as pso:

            # ---- PE p-state warm-up (independent of inputs) ----
            wsc = const.tile([128, 512], BF16, tag="wsc")
            nc.gpsimd.memset(wsc[:], 0.5)
            for _w in range(6):
                wps = ps.tile([128, 1024], F32, tag="pab")
                nc.tensor.matmul(wps[:, 0:512], wsc[:, 0:128], wsc[:],
                                 start=True, stop=True)

            # ---- inputs; head 0 split fine-grained so chunk 0 starts early,
            # qt on the Activation DGE queue to parallelize the ramp ----
            qts, kts, vts = [], [], []
            for h in range(VH):
                kt = inp.tile([128, S], BF16, tag=f"kt{h}")
                qt = inp.tile([128, S], BF16, tag=f"qt{h}")
                vt = inp.tile([128, S], BF16, tag=f"vt{h}")
                qts.append(qt)
                kts.append(kt)
                vts.append(vt)
            lam = inp.tile([1, 4 * D], F32, tag="lam")
            wqr = inp.tile([1, VH * 128], F32, tag="wqr")
            bqr = inp.tile([1, VH * 128], F32, tag="bqr")
            # SP queue: head-0 k/v first (split so chunk 0 starts early),
            # then the small param rows, then head 1.
            nc.sync.dma_start(kts[0][:, 0:128], kt_d[0][:, 0:128])
            nc.sync.dma_start(kts[0][:, 128:512], kt_d[0][:, 128:512])
            nc.sync.dma_start(vts[0][:, 0:512], vt_d[0][:, 0:512])
            nc.sync.dma_start(wqr[:], wq_d[:])
            nc.sync.dma_start(bqr[:], bq_d[:])
            nc.sync.dma_start(kts[0][:, 512:S], kt_d[0][:, 512:S])
            nc.sync.dma_start(vts[0][:, 512:S], vt_d[0][:, 512:S])
            nc.sync.dma_start(kts[1][:], kt_d[1])
            nc.sync.dma_start(vts[1][:], vt_d[1])
            # Activation queue (parallel): q and lambda
            nc.scalar.dma_start(qts[0][:, 0:512], qt_d[0][:, 0:512])
            nc.scalar.dma_start(lam[:], lam_d[:])
            nc.scalar.dma_start(qts[0][:, 512:S], qt_d[0][:, 512:S])
            nc.scalar.dma_start(qts[1][:], qt_d[1])

            invlamv = const.tile([128, 1], F32, tag="invlamv")
            wqb, bqb = [], []
            for h in range(VH):
                wb = const.tile([128, 128], F32, tag=f"wqb{h}")
                bb = const.tile([128, 128], F32, tag=f"bqb{h}")
                wqb.append(wb)
                bqb.append(bb)

            def make_prep():
                def prep():
                    # lambda_full = exp(lq1.lk1) - exp(lq2.lk2) + 0.8
                    scr = sm.tile([1, D], F32, tag="lscr")
                    s12 = sm.tile([1, 2], F32, tag="ls12")
                    nc.vector.tensor_tensor(scr[:], lam[:, 0:D],
                                            lam[:, D:2 * D], ALU.mult)
                    nc.vector.tensor_reduce(s12[:, 0:1], scr[:],
                                            mybir.AxisListType.X, ALU.add)
                    nc.vector.tensor_tensor(scr[:], lam[:, 2 * D:3 * D],
                                            lam[:, 3 * D:4 * D], ALU.mult)
                    nc.vector.tensor_reduce(s12[:, 1:2], scr[:],
                                            mybir.AxisListType.X, ALU.add)
                    e12 = sm.tile([1, 2], F32, tag="le12")
                    nc.scalar.activation(e12[:], s12[:], AF.Exp)
                    lamf = sm.tile([1, 1], F32, tag="lamf")
                    nc.vector.tensor_tensor(lamf[:], e12[:, 0:1], e12[:, 1:2],
                                            ALU.subtract)
                    nc.vector.tensor_scalar(lamf[:], lamf[:], LAMBDA_INIT,
                                            None, ALU.add)
                    rlamf = sm.tile([1, 1], F32, tag="rlamf")
                    nc.vector.reciprocal(rlamf[:], lamf[:])
                    nc.gpsimd.partition_broadcast(invlamv[:], rlamf[:])
                    for h in range(VH):
                        nc.gpsimd.partition_broadcast(
                            wqb[h][:], wqr[:, h * 128:(h + 1) * 128])
                        nc.gpsimd.partition_broadcast(
                            bqb[h][:], bqr[:, h * 128:(h + 1) * 128])
                return prep

            octs, sums, sqs, zs = [], [], [], []
            for h in range(VH):
                oct_t = inp.tile([128, S], BF16, tag=f"oct{h}")
                sums_t = inp.tile([128, NPASS], F32, tag=f"sums{h}")
                sqs_t = inp.tile([128, NPASS], F32, tag=f"sqs{h}")
                z_t = inp.tile([128, S], BF16, tag=f"z{h}")
                octs.append(oct_t)
                sums.append(sums_t)
                sqs.append(sqs_t)
                zs.append(z_t)

            def make_epilogue(h, qp, acc, o01):
                qsl = slice(qp * QP, (qp + 1) * QP)

                dts = []

                def epi_a():
                    # column sums per map half on Pool (d0 first, so its
                    # reciprocal can overlap d1's all_reduce); fold 1/lambda
                    # into acc's d1 half so r1 = lambda/d1
                    dt = dp.tile([128, 2 * QP], F32, tag="dt")
                    rt = dp.tile([128, 2 * QP], F32, tag="rt")
                    dts.append((dt, rt))
                    nc.gpsimd.partition_all_reduce(dt[:, 0:QP], acc[:, 0:QP],
                                                   128, RED.add)
                    nc.vector.tensor_scalar(acc[:, QP:2 * QP],
                                            acc[:, QP:2 * QP],
                                            invlamv[:], None, ALU.mult)
                    nc.gpsimd.partition_all_reduce(dt[:, QP:2 * QP],
                                                   acc[:, QP:2 * QP],
                                                   128, RED.add)

                def epi_b():
                    dt, rt = dts[0]
                    t0 = work.tile([128, QP], F32, tag="t0")
                    nc.vector.reciprocal(rt[:, 0:QP], dt[:, 0:QP])
                    nc.vector.tensor_tensor(t0[:], o01[:, 0:QP],
                                            rt[:, 0:QP], ALU.mult)
                    dts.append(t0)

                def epi_c():
                    dt, rt = dts[0]
                    t0 = dts[1]
                    t1 = work.tile([128, QP], F32, tag="t1")
                    nc.vector.reciprocal(rt[:, QP:2 * QP], dt[:, QP:2 * QP])
                    nc.vector.tensor_tensor(t1[:], o01[:, QP:2 * QP],
                                            rt[:, QP:2 * QP], ALU.mult)
                    nc.vector.scalar_tensor_tensor(
                        octs[h][:, qsl], t0[:], 1.0, t1[:],
                        ALU.mult, ALU.subtract,
                        accum_out=sums[h][:, qp:qp + 1])

                def epi_d():
                    scr2 = work.tile([128, QP], BF16, tag="scr2")
                    nc.vector.scalar_tensor_tensor(
                        scr2[:], octs[h][:, qsl], 1.0, octs[h][:, qsl],
                        ALU.mult, ALU.mult,
                        accum_out=sqs[h][:, qp:qp + 1])

                def epi_z():
                    # z = oct * w[q>>4]; lets the final apply be one fused op
                    nc.gpsimd.tensor_tensor(
                        zs[h][:, qsl].rearrange("p (c s) -> p c s", c=32),
                        octs[h][:, qsl].rearrange("p (c s) -> p c s", c=32),
                        wqb[h][:, qp * 32:(qp + 1) * 32]
                            .rearrange("p (c one) -> p c one", one=1)
                            .broadcast_to([128, 32, 16]),
                        ALU.mult)

                return [epi_a, epi_b, epi_c, epi_z, epi_d]

            def make_gn(h):
                def gn_stats():
                    st = sm.tile([128, 4], F32, tag="st")
                    nc.vector.tensor_reduce(st[:, 0:1], sums[h][:],
                                            mybir.AxisListType.X, ALU.add)
                    nc.vector.tensor_reduce(st[:, 1:2], sqs[h][:],
                                            mybir.AxisListType.X, ALU.add)
                    nc.gpsimd.partition_all_reduce(st[:, 0:2], st[:, 0:2],
                                                   128, RED.add)
                    mu = sm.tile([128, 4], F32, tag="mu")
                    nc.vector.tensor_scalar(mu[:, 0:1], st[:, 0:1], inv_n,
                                            None, ALU.mult)            # mu
                    nc.vector.tensor_scalar(mu[:, 1:2], st[:, 1:2], inv_n,
                                            EPS, ALU.mult, ALU.add)  # E2+eps
                    nc.vector.tensor_scalar(mu[:, 3:4], mu[:, 0:1], -1.0,
                                            None, ALU.mult)            # -mu
                    nc.vector.scalar_tensor_tensor(
                        mu[:, 2:3], mu[:, 0:1], mu[:, 3:4], mu[:, 1:2],
                        ALU.mult, ALU.add)       # var+eps = E2+eps - mu^2
                    # 1/sigma: magic-constant rsqrt + 2 Newton steps on DVE
                    sh = sm.tile([128, 1], I32, tag="sh")
                    nc.vector.tensor_scalar(sh[:], mu[:, 2:3].bitcast(I32),
                                            1, None, ALU.logical_shift_right)
                    nc.vector.tensor_scalar(sh[:], sh[:], -1, MAGIC,
                                            ALU.mult, ALU.add)
                    y = sm.tile([128, 1], F32, tag="y")
                    nc.vector.tensor_copy(y[:].bitcast(I32), sh[:])
                    t = sm.tile([128, 1], F32, tag="t")
                    hna = sm.tile([128, 1], F32, tag="hna")
                    nc.vector.tensor_scalar(hna[:], mu[:, 2:3], -0.5, None,
                                            ALU.mult)
                    for _ in range(2):
                        nc.vector.tensor_tensor(t[:], y[:], y[:], ALU.mult)
                        nc.vector.tensor_scalar(t[:], t[:], hna[:], 1.5,
                                                ALU.mult, ALU.add)
                        nc.vector.tensor_tensor(y[:], y[:], t[:], ALU.mult)
                    i02 = sm.tile([128, 1], F32, tag="i02")
                    nc.vector.tensor_scalar(i02[:], y[:],
                                            1.0 - LAMBDA_INIT, None, ALU.mult)
                    # C = b*0.2 - mu*i02*w  (z*i02 + C is the full apply)
                    nmi = sm.tile([128, 1], F32, tag="nmi")
                    nc.vector.tensor_tensor(nmi[:], mu[:, 3:4], i02[:],
                                            ALU.mult)
                    cc = sm.tile([128, 128], F32, tag=f"cc{h}")
                    nc.vector.scalar_tensor_tensor(
                        cc[:], wqb[h][:], nmi[:], bqb[h][:],
                        ALU.mult, ALU.add)
                    return i02, cc

                coeffs = []
                pieces = [lambda: coeffs.append(gn_stats())]

                # out = z * (1/sigma * 0.2) + C[q>>4]: one fused op per
                # quarter, DMAs overlap the remaining applies
                outf = inp.tile([128, S], BF16, tag=f"outf{h}")

                def make_quarter(qu):
                    def quarter():
                        i02, cc = coeffs[0]
                        qs = slice(qu * (S // 4), (qu + 1) * (S // 4))
                        qa = slice(qu * 32, (qu + 1) * 32)
                        nc.vector.scalar_tensor_tensor(
                            outf[:, qs].rearrange("p (c s) -> p c s", c=32),
                            zs[h][:, qs].rearrange("p (c s) -> p c s", c=32),
                            i02[:],
                            cc[:, qa].rearrange("p (c one) -> p c one", one=1)
                                .broadcast_to([128, 32, 16]),
                            ALU.mult, ALU.add)
                        nc.sync.dma_start(out_d[h, :, qs], outf[:, qs])
                    return quarter

                for qu in range(4):
                    pieces.append(make_quarter(qu))
                return pieces

            # ---- main pipeline: flat over 128 global chunk steps ----
            passes = [(h, qp) for h in range(VH) for qp in range(NPASS)]
            accs = {}
            o01s = {}
            eabs = {}
            pending = [make_prep()]  # queue of small emitters, 1 per step

            def emit_scores_exp_acc(g):
                p, c = g // NCH, g % NCH
                h, qp = passes[p]
                qsl = slice(qp * QP, (qp + 1) * QP)
                csl = slice(c * 128, (c + 1) * 128)
                if c == 0:
                    acc = accp.tile([128, 2 * QP], FP16, tag="acc")
                    o01 = pso.tile([128, 2 * QP], F32, tag="o01")
                    accs[p] = acc
                    o01s[p] = o01
                pab = ps.tile([128, 2 * QP], F32, tag="pab")
                nc.tensor.matmul(pab[:, 0:QP], kts[h][0:64, csl],
                                 qts[h][0:64, qsl], start=True, stop=True)
                nc.tensor.matmul(pab[:, QP:2 * QP], kts[h][64:128, csl],
                                 qts[h][64:128, qsl], start=True, stop=True)
                eab = eabp.tile([128, 2 * QP], BF16, tag="eab")
                nc.scalar.activation(eab[:], pab[:], AF.Exp, scale=SCALE)
                eabs[g] = eab
                acc = accs[p]
                if c == 0:
                    # ghostmax: +1/128 per partition carries the +1
                    nc.vector.tensor_scalar(acc[:], eab[:], 1.0 / 128.0,
                                            None, ALU.add)
                else:
                    nc.vector.tensor_tensor(acc[:], acc[:], eab[:], ALU.add)

            def emit_av(g):
                p, c = g // NCH, g % NCH
                h, _ = passes[p]
                csl = slice(c * 128, (c + 1) * 128)
                o01 = o01s[p]
                eab = eabs.pop(g)
                nc.tensor.matmul(o01[:, 0:QP], vts[h][:, csl], eab[:, 0:QP],
                                 start=(c == 0), stop=(c == NCH - 1))
                nc.tensor.matmul(o01[:, QP:2 * QP], vts[h][:, csl],
                                 eab[:, QP:2 * QP],
                                 start=(c == 0), stop=(c == NCH - 1))

            for g in range(G + 2):
                if g < G:
                    emit_scores_exp_acc(g)
                if g >= 2:
                    emit_av(g - 2)
                    gp = g - 2
                    if gp % NCH == NCH - 1:
                        p = gp // NCH
                        h, qp = passes[p]
                        pending.extend(
                            make_epilogue(h, qp, accs.pop(p), o01s.pop(p)))
                        if qp == NPASS - 1:
                            pending.extend(make_gn(h))
                    if pending:
                        pending.pop(0)()
            for f in pending:
                f()

    nc.finalize()
    return nc


def _get_program():
    global _PROGRAM
    if _PROGRAM is None:
        _PROGRAM = _build_program()
    return _PROGRAM


def _prepare_in_maps(q, k, v, lambda_q1, lambda_k1, lambda_q2, lambda_k2,
                     gn_weight, gn_bias):
    q = np.asarray(q)
    k = np.asarray(k)
    v = np.asarray(v)

    lam = np.concatenate([np.asarray(lambda_q1), np.asarray(lambda_k1),
                          np.asarray(lambda_q2), np.asarray(lambda_k2)]
                         ).astype(np.float32).reshape(1, 4 * D)
    w_hq = np.asarray(gn_weight, dtype=np.float32).reshape(HQ, 128)
    b_hq = np.asarray(gn_bias, dtype=np.float32).reshape(HQ, 128) \
        * (1.0 - LAMBDA_INIT)

    in_maps = []
    for core in range(NCORE):
        heads = [core * VH + i for i in range(VH)]
        qt = np.empty((VH, 128, S), dtype=ml_dtypes.bfloat16)
        kt = np.empty((VH, 128, S), dtype=ml_dtypes.bfloat16)
        vt = np.empty((VH, 128, S), dtype=ml_dtypes.bfloat16)
        wq = np.empty((1, VH * 128), dtype=np.float32)
        bq = np.empty((1, VH * 128), dtype=np.float32)
        for i, hh in enumerate(heads):
            qt[i, 0:64] = q[0, 2 * hh].T.astype(ml_dtypes.bfloat16)
            qt[i, 64:128] = q[0, 2 * hh + 1].T.astype(ml_dtypes.bfloat16)
            kt[i, 0:64] = k[0, 2 * hh].T.astype(ml_dtypes.bfloat16)
            kt[i, 64:128] = k[0, 2 * hh + 1].T.astype(ml_dtypes.bfloat16)
            vt[i] = (v[0, hh].reshape(NCH, 128, DV).transpose(1, 0, 2)
                     .reshape(128, S).astype(ml_dtypes.bfloat16))
            wq[0, i * 128:(i + 1) * 128] = w_hq[hh]
            bq[0, i * 128:(i + 1) * 128] = b_hq[hh]
        in_maps.append({"qt": qt, "kt": kt, "vt": vt, "lam": lam,
                        "wq": wq, "bq": bq})
    return in_maps


def _assemble(results):
    # device out[h] = [dv, s]; head output is [s, dv]
    out_heads = np.empty((HQ, S, DV), dtype=np.float32)
    for core in range(NCORE):
        o = results[core]["out"]                      # [VH, 128, 2048] bf16
        for i in range(VH):
            out_heads[core * VH + i] = np.asarray(o[i]).astype(np.float32).T
    x = out_heads.reshape(HQ * DV, S)                 # torch-style flatten
    return np.ascontiguousarray(x.T)[None]            # [1, S, C]


def kernel(**inputs):
    nc = _get_program()
    in_maps = _prepare_in_maps(**inputs)
    res = run_bass_kernel_spmd(nc, in_maps, list(range(NCORE)))
    return _assemble(res.results)
